# revision 67
# baseline (speedup 1.0000x reference)
"""Trainium2 Bass kernel for a 2-layer GAT occupancy predictor (B=1).

Reference math:
  pts = concat(pos, pos_non_manifold) -> [K=6000, 3]
  mask[i,j] = ||pts_i - pts_j||^2 < 0.05^2          (dense radius graph)
  layer l:  h = x @ Wl                              [K, 4*64]
            e[i,j,h] = leaky02(ed[i,h] + es[j,h])   es/ed = <h, a_src/dst>
            alpha = softmax_j(e masked)
            x' = relu(alpha @ h + b)
  logits = (x2 @ fc_w + fc_b)[M:] reshaped to [1, 2, 3000]

Distribution (8 NeuronCores): nodes are Morton-sorted on the host so the
radius graph becomes block-local; core c owns destination rows
[768c, 768(c+1)) of the sorted, padded 6144-node graph.

Layer 1 is fully static and runs per destination chunk: for each of the 6
own 128-node chunks the host compacts the ~200 in-radius source nodes into
2-3 tiles (padded per chunk position across cores), and the score pipeline
runs merged over the chunk's tiles and all 4 heads via stride-0 broadcast
APs (one add, one 0.2x, one max, one mask-add, one exp per chunk).

Layer 2 runs per whole-core source slot: the ~870 unique sources are
compacted into 8 slots of 128 ordered [own 768 | halo | pad].  Each core's
own nodes are permuted boundary-first so the single bf16 AllGather between
layers only ships the rows other cores fetch; layer 2 computes own-slot h
from the local transposed features and fetches the halo slots' x rows with
one indirect DMA per slot, transposing on the PE.  The halo chain is
pushed late in the static schedule (tile_set_cur_wait) so the in-order
engine queues don't stall on the collective.  Layer-2 slot masks are
computed from static coordinates during the inter-layer window, filling
the AllGather bubble.

Engine mapping:
  PE   : h (with es riding along as 4 extra host-folded weight columns),
         d2 = |p_i - p_j|^2 as K=5 matmuls ([p; sq; 1] x [-2p; 1; sq]),
         the alpha @ h aggregation as bf16 hi+lo pairs into shared PSUM
         banks, denominators as N=1 ones-column matmuls.
  DVE  : scores, leaky, mask-adds (broadcast tensor_tensor), h hi/lo
         split, finalize division/relu, fc.
  ACT  : layer-2 head-0 leaky via Prelu, exp, PSUM->SBUF copies.
  Pool : partition-broadcast of ed, indirect halo gathers, AllGather.
Mask offsets (-30/0 bf16): per-pair tiles in layer 1, SBUF-resident
per-slot planes for layer 2.
Padded nodes sit at (-1,-1,-1): finite features, outside every real radius.
"""

import sys

sys.path.insert(0, "/opt/trn_rl_repo")

from contextlib import ExitStack

import ml_dtypes
import numpy as np

import concourse.bacc as bacc
import concourse.bass as bass
import concourse.mybir as mybir
import concourse.tile as tile
from concourse.bass_utils import run_bass_kernel_spmd

F32 = mybir.dt.float32
BF16 = mybir.dt.bfloat16
I32 = mybir.dt.int32
AF = mybir.ActivationFunctionType
OP = mybir.AluOpType
AX = mybir.AxisListType

N_CORES = 8
N = 3000
M = 3000
K = N + M          # real nodes
KP = 6144          # padded nodes
NT = KP // 128     # 48
IC = KP // N_CORES # 768 destinations per core
ICT = IC // 128    # 6 destination chunks per core
OWN_SLOTS = ICT    # first 6 layer-2 slots are the core's own nodes
H = 4              # heads
C = 64             # channels per head
HC = H * C         # 256
HCE = HC + H       # h columns + es columns
CP1 = C + 1        # head channels + ones column
R2 = float(np.float32(0.05) * np.float32(0.05))
PAD_COORD = -1.0
MASK_EPS = 1e-5    # host activity-test margin (superset of device mask)
MNEG = -30.0       # masked-score offset: exp(-30+L) ~ 1e-12
PAD_NODE = KP - 1  # all-padding node, used for unused slot entries


def build(nslot, bp, t1, n_cores=N_CORES, fake_ag=False):
    nc = bacc.Bacc("TRN2", target_bir_lowering=False, debug=False,
                   num_devices=n_cores)
    NS = nslot
    NH = NS - OWN_SLOTS          # halo slots (gathered in layer 2)
    BP = bp                      # boundary rows shipped in the AllGather
    NT1 = sum(t1)                # layer-1 chunk-tile count
    T1MAX = max(t1)
    assert NH >= 1 and BP % 128 == 0 and len(t1) == ICT

    # ---- kernel I/O (identical program on every core) ----
    # *sel5 rows: [p(3); sq; ones] for sources
    # pts_own5 rows: [-2p(3); ones; sq] for the own destination columns
    pts_sel5_d = nc.dram_tensor("pts_sel5", [5, NS * 128], F32,
                                kind="ExternalInput")
    pts1_d = nc.dram_tensor("pts1", [5, NT1 * 128], F32,
                            kind="ExternalInput")
    pts_own5_d = nc.dram_tensor("pts_own5", [5, IC], F32,
                                kind="ExternalInput")
    pts_own3_d = nc.dram_tensor("pts_own3", [3, IC], F32,
                                kind="ExternalInput")
    agidx_d = nc.dram_tensor("agidx", [128, NH], I32, kind="ExternalInput")
    # w1p/w2p: [W | W @ a_src_blockdiag] so es rides along with h
    w1p_d = nc.dram_tensor("w1p", [3, HCE], F32, kind="ExternalInput")
    w2p_d = nc.dram_tensor("w2p", [HC, HCE], BF16, kind="ExternalInput")
    adm1_d = nc.dram_tensor("adm1", [HC, H], BF16, kind="ExternalInput")
    adm2_d = nc.dram_tensor("adm2", [HC, H], BF16, kind="ExternalInput")
    bias1_d = nc.dram_tensor("bias1", [128, HC], F32, kind="ExternalInput")
    bias2_d = nc.dram_tensor("bias2", [128, HC], F32, kind="ExternalInput")
    fcw_d = nc.dram_tensor("fcw", [128, 2 * HC], F32, kind="ExternalInput")
    fcb_d = nc.dram_tensor("fcb", [128, 2], F32, kind="ExternalInput")
    ident_d = nc.dram_tensor("ident", [128, 128], F32, kind="ExternalInput")

    out_d = nc.dram_tensor("out", [IC, 2], F32, kind="ExternalOutput")

    with tile.TileContext(nc) as tc, ExitStack() as st:
        dram = st.enter_context(tc.tile_pool(name="dram", bufs=1,
                                             space="DRAM"))
        x_bounce = dram.tile([BP, HC], BF16)
        ag_out = dram.tile([n_cores * BP, HC], BF16,
                           addr_space=("Local" if fake_ag else "Shared"))

        const = st.enter_context(tc.tile_pool(name="const", bufs=1))
        pts_sel5_sb = const.tile([5, NS * 128], F32)
        pts1_sb = const.tile([5, NT1 * 128], F32)
        pts_own5_sb = const.tile([5, IC], F32)
        pts_own3_sb = const.tile([3, IC], F32)
        agidx_sb = const.tile([128, NH], I32)
        w1p_sb = const.tile([3, HCE], F32)
        w2p_sb = const.tile([128, 2, HCE], BF16)
        adm1_sb = const.tile([128, 2, H], BF16)
        adm2_sb = const.tile([128, 2, H], BF16)
        bias1_sb = const.tile([128, HC], F32)
        bias2_sb = const.tile([128, HC], F32)
        fcw_sb = const.tile([128, 2 * HC], F32)
        fcb_sb = const.tile([128, 2], F32)
        ident_sb = const.tile([128, 128], F32)
        ident_bf = const.tile([128, 128], BF16)
        w1b = const.tile([3, HCE], BF16)
        own3b = const.tile([3, IC], BF16)

        nc.sync.dma_start(out=pts_sel5_sb[:, :], in_=pts_sel5_d[:, :])
        nc.sync.dma_start(out=pts1_sb[:, :], in_=pts1_d[:, :])
        nc.sync.dma_start(out=pts_own5_sb[:, :], in_=pts_own5_d[:, :])
        nc.sync.dma_start(out=pts_own3_sb[:, :], in_=pts_own3_d[:, :])
        nc.sync.dma_start(out=agidx_sb[:, :], in_=agidx_d[:, :])
        nc.sync.dma_start(out=w1p_sb[:, :], in_=w1p_d[:, :])
        nc.sync.dma_start(out=w2p_sb[:, :, :],
                          in_=w2p_d.rearrange("(s p) c -> p s c", p=128))
        nc.sync.dma_start(out=adm1_sb[:, :, :],
                          in_=adm1_d.rearrange("(s p) h -> p s h", p=128))
        nc.sync.dma_start(out=adm2_sb[:, :, :],
                          in_=adm2_d.rearrange("(s p) h -> p s h", p=128))
        nc.sync.dma_start(out=bias1_sb[:, :], in_=bias1_d[:, :])
        nc.sync.dma_start(out=bias2_sb[:, :], in_=bias2_d[:, :])
        nc.sync.dma_start(out=fcw_sb[:, :], in_=fcw_d[:, :])
        nc.sync.dma_start(out=fcb_sb[:, :], in_=fcb_d[:, :])
        nc.sync.dma_start(out=ident_sb[:, :], in_=ident_d[:, :])
        nc.scalar.copy(ident_bf[:, :], ident_sb[:, :])
        nc.scalar.copy(w1b[:, :], w1p_sb[:, :])
        nc.scalar.copy(own3b[:, :], pts_own3_sb[:, :])

        big = st.enter_context(tc.tile_pool(name="big", bufs=1))
        hp1_hi = big.tile([128, NT1, H, CP1], BF16)
        hp1_lo = big.tile([128, NT1, H, CP1], BF16)
        es41 = big.tile([128, NT1, H], BF16)
        hp_hi = big.tile([128, NS, H, CP1], BF16)
        hp_lo = big.tile([128, NS, H, CP1], BF16)
        es4 = big.tile([128, NS, H], F32)
        ed_b = big.tile([128, H, IC], BF16)
        x_sb = big.tile([128, ICT, HC], F32)
        x_bf = big.tile([128, ICT, HC], BF16)
        xt_own = big.tile([128, 2, IC], BF16)
        edt_sb = big.tile([H, IC], BF16)
        edt_rows = big.tile([1, H, IC], BF16)
        logit_sb = big.tile([128, ICT, 2], F32)
        mn_sb = big.tile([128, NS, IC], BF16)   # layer-2 slot mask offsets

        nc.vector.memset(hp1_hi[:, :, :, C:CP1], 1.0)
        nc.vector.memset(hp1_lo[:, :, :, C:CP1], 0.0)
        nc.vector.memset(hp_hi[:, :, :, C:CP1], 1.0)
        nc.vector.memset(hp_lo[:, :, :, C:CP1], 0.0)

        # ================= layer 1 (per destination chunk) ==============
        # ---- own-column side: hT(own), edT, ED broadcasts ----
        with tc.tile_pool(name="prep1", bufs=2) as prep, \
             tc.tile_pool(name="prep_ps1", bufs=1, space="PSUM") as prep_ps:
            ht_own = prep.tile([128, 2, IC], BF16, tag="ht", bufs=1)
            for oc in range(2):
                ht_ps = prep_ps.tile([128, IC], F32, tag="ht_ps", bufs=1,
                                     name=f"ht_ps_1_{oc}")
                for lo, sz in ((0, 512), (512, 256)):
                    sl = slice(lo, lo + sz)
                    nc.tensor.matmul(
                        ht_ps[:, sl], w1b[:, oc * 128:(oc + 1) * 128],
                        own3b[:, sl], start=True, stop=True)
                nc.scalar.copy(ht_own[:, oc, :], ht_ps[:, :])

            edt_ps = prep_ps.tile([H, IC], F32, tag="edt", bufs=1)
            for s in range(2):
                for lo, sz in ((0, 512), (512, 256)):
                    sl = slice(lo, lo + sz)
                    nc.tensor.matmul(edt_ps[:, sl], adm1_sb[:, s, :],
                                     ht_own[:, s, sl],
                                     start=(s == 0), stop=(s == 1))
            nc.scalar.copy(edt_sb[:, :], edt_ps[:, :])
            for h in range(H):
                nc.sync.dma_start(out=edt_rows[0:1, h, :],
                                  in_=edt_sb[h:h + 1, :])
            # stagger: first chunks' columns land before the full rows so
            # the chunk loop can start scoring sooner
            for h in range(H):
                nc.gpsimd.partition_broadcast(ed_b[:, h, 0:256],
                                              edt_rows[0:1, h, 0:256])
            for h in range(H):
                nc.gpsimd.partition_broadcast(ed_b[:, h, 256:IC],
                                              edt_rows[0:1, h, 256:IC])

        # ---- chunk loop: h+es, per-pair masks, merged scores, agg ----
        with tc.tile_pool(name="agg_ps1", bufs=1, space="PSUM") as agg_ps:
            aggp = [agg_ps.tile([128, 2, H, C], F32, tag=f"agg{p}",
                                name=f"agg_1_{p}")
                    for p in range(ICT // 2)]
            den0 = agg_ps.tile([128, 2, H], F32, tag="den0",
                               name="den0_1")
            den2 = agg_ps.tile([128, ICT - 2, H], F32, tag="den2",
                               name="den2_1")
            with tc.tile_pool(name="jl1", bufs=3) as jl, \
                 tc.tile_pool(name="h_ps1", bufs=1, space="PSUM") as h_psp:
                # pass 1: per-tile h/es/hi-lo/masks (independent of ed_b,
                # fills the DVE queue while the prep broadcasts run)
                mnCs = []
                tbase = 0
                for ci in range(ICT):
                    Tc = t1[ci]
                    csl = slice(ci * 128, (ci + 1) * 128)
                    mnC = jl.tile([128, T1MAX, 128], BF16, tag="mn",
                                  bufs=ICT, name=f"mn1_{ci}")
                    mnCs.append(mnC)
                    for t in range(Tc):
                        s = tbase + t
                        ssl = slice(s * 128, (s + 1) * 128)
                        h_ps = h_psp.tile([128, HCE], F32, tag="h",
                                          name=f"h1_{s}")
                        nc.tensor.matmul(h_ps[:, :], pts1_sb[0:3, ssl],
                                         w1p_sb[:, :],
                                         start=True, stop=True)
                        nc.vector.tensor_scalar_add(
                            es41[:, s, :], h_ps[:, HC:HCE], 0.0)
                        nc.scalar.copy(
                            hp1_hi[:, s, :, 0:C],
                            h_ps[:, 0:HC].rearrange("p (h c) -> p h c",
                                                    h=H))
                        nc.vector.tensor_tensor(
                            hp1_lo[:, s, :, 0:C],
                            h_ps[:, 0:HC].rearrange("p (h c) -> p h c",
                                                    h=H),
                            hp1_hi[:, s, :, 0:C], OP.subtract)
                        g_ps = h_psp.tile([128, 128], F32, tag="g",
                                          bufs=2, name=f"g1_{s}")
                        nc.tensor.matmul(g_ps[:, :], pts1_sb[:, ssl],
                                         pts_own5_sb[:, csl],
                                         start=True, stop=True)
                        nc.vector.tensor_scalar(
                            mnC[:, t, :], g_ps[:, :], R2, MNEG,
                            OP.is_ge, OP.mult)
                    tbase += Tc

                # pass 2: merged scores + aggregation per chunk
                tbase = 0
                for ci in range(ICT):
                    Tc = t1[ci]
                    csl = slice(ci * 128, (ci + 1) * 128)
                    mnC = mnCs[ci]
                    L = jl.tile([128, Tc, H, 128], BF16, tag="L4",
                                name=f"L1_{ci}")
                    e1 = ed_b[:, :, csl].rearrange(
                        "p (o h) d -> p o h d", o=1)
                    e2 = es41[:, tbase:tbase + Tc, :].rearrange(
                        "p t (h o) -> p t h o", o=1)
                    b1, b2 = bass.broadcast_tensor_aps(e1, e2)
                    nc.vector.tensor_tensor(L[:, :, :, :], b1, b2, OP.add)
                    T4 = jl.tile([128, Tc, H, 128], BF16, tag="T4",
                                 name=f"T4_1_{ci}")
                    nc.vector.tensor_scalar(T4[:, :, :, :], L[:, :, :, :],
                                            0.2, None, OP.mult)
                    nc.vector.tensor_tensor(L[:, :, :, :], L[:, :, :, :],
                                            T4[:, :, :, :], OP.max)
                    mb = mnC[:, 0:Tc, :].rearrange(
                        "p t (o d) -> p t o d", o=1)
                    bl, bm = bass.broadcast_tensor_aps(L[:, :, :, :], mb)
                    nc.vector.tensor_tensor(L[:, :, :, :], bl, bm, OP.add)
                    A = jl.tile([128, Tc, H, 128], BF16, tag="A4",
                                name=f"A1_{ci}")
                    if ci == ICT - 1:
                        nc.scalar.activation(A[:, 0:1, :, :],
                                             L[:, 0:1, :, :], AF.Exp)
                        nc.scalar.activation(A[:, 1:Tc, :, :],
                                             L[:, 1:Tc, :, :], AF.Exp)
                    else:
                        nc.scalar.activation(A[:, :, :, :], L[:, :, :, :],
                                             AF.Exp)

                    bank = aggp[ci // 2]
                    half = ci % 2
                    for t in range(Tc):
                        s = tbase + t
                        for h in range(H):
                            first = (half == 0 and t == 0 and h == 0)
                            last = (half == 1 and t == Tc - 1
                                    and h == H - 1)
                            nc.tensor.matmul(
                                bank[:, half, h, :], A[:, t, h, :],
                                hp1_hi[:, s, h, 0:C],
                                start=first, stop=False)
                            nc.tensor.matmul(
                                bank[:, half, h, :], A[:, t, h, :],
                                hp1_lo[:, s, h, 0:C],
                                start=False, stop=last)
                            if ci < 2:
                                nc.tensor.matmul(
                                    den0[:, ci, h:h + 1], A[:, t, h, :],
                                    hp1_hi[:, s, h, C:CP1],
                                    start=(ci == 0 and t == 0 and h == 0),
                                    stop=(ci == 1 and t == Tc - 1
                                          and h == H - 1))
                            else:
                                nc.tensor.matmul(
                                    den2[:, ci - 2, h:h + 1],
                                    A[:, t, h, :],
                                    hp1_hi[:, s, h, C:CP1],
                                    start=(ci == 2 and t == 0 and h == 0),
                                    stop=(ci == ICT - 1 and t == Tc - 1
                                          and h == H - 1))
                    # boundary chunks complete: finalize them and fire the
                    # AllGather mid-layer so the barrier+data overlap the
                    # remaining chunks
                    if ci == 1:
                        for ic in range(2):
                            rec = jl.tile([128, H], F32, tag="rec0",
                                          name=f"rec_e_{ic}")
                            nc.vector.reciprocal(rec[:, :],
                                                 den0[:, ic, :])
                            for h in range(H):
                                nc.vector.scalar_tensor_tensor(
                                    x_sb[:, ic, h * C:(h + 1) * C],
                                    aggp[0][:, ic, h, :],
                                    rec[:, h:h + 1],
                                    bias1_sb[:, h * C:(h + 1) * C],
                                    OP.mult, OP.add)
                        nc.vector.tensor_scalar(
                            x_sb[:, 0:2, :], x_sb[:, 0:2, :],
                            0.0, None, OP.max)
                        nc.scalar.copy(x_bf[:, 0:2, :], x_sb[:, 0:2, :])
                        nc.sync.dma_start(
                            out=x_bounce.rearrange("(q p) c -> p q c",
                                                   p=128),
                            in_=x_bf[:, 0:2, :])
                        if fake_ag:
                            for r in range(n_cores):
                                nc.sync.dma_start(
                                    out=ag_out[r * BP:(r + 1) * BP, :],
                                    in_=x_bounce[:, :])
                        else:
                            nc.gpsimd.collective_compute(
                                "AllGather", OP.bypass,
                                replica_groups=[list(range(n_cores))],
                                ins=[x_bounce.opt()],
                                outs=[ag_out.opt()])
                    tbase += Tc

            # ---- finalize x1, AllGather, transposes, layer-2 masks ----
            with tc.tile_pool(name="fin1", bufs=2) as fin, \
                 tc.tile_pool(name="fin_ps1", bufs=1,
                              space="PSUM") as fin_ps:
                for ic in range(2, ICT):
                    rec = fin.tile([128, H], F32, tag="rec",
                                   name=f"rec_1_{ic}")
                    nc.vector.reciprocal(rec[:, :], den2[:, ic - 2, :])
                    for h in range(H):
                        nc.vector.scalar_tensor_tensor(
                            x_sb[:, ic, h * C:(h + 1) * C],
                            aggp[ic // 2][:, ic % 2, h, :],
                            rec[:, h:h + 1],
                            bias1_sb[:, h * C:(h + 1) * C],
                            OP.mult, OP.add)
                nc.vector.tensor_scalar(x_sb[:, 2:, :], x_sb[:, 2:, :],
                                        0.0, None, OP.max)
                # transposed own x, needed for layer-2 prep
                for ic in range(ICT):
                    for oc in range(2):
                        t_ps = fin_ps.tile([128, 128], F32, tag="t_ps",
                                           name=f"t_ps_{ic}_{oc}")
                        nc.tensor.transpose(
                            t_ps[:, :],
                            x_sb[:, ic, oc * 128:(oc + 1) * 128],
                            ident_sb[:, :])
                        nc.scalar.copy(
                            xt_own[:, oc, ic * 128:(ic + 1) * 128],
                            t_ps[:, :])
                # layer-2 slot masks from static coords: fills the
                # AllGather bubble
                with tc.tile_pool(name="mn_ps2", bufs=1,
                                  space="PSUM") as mn_psp:
                    for s in range(NS):
                        for lo, sz in ((0, 512), (512, 256)):
                            sl = slice(lo, lo + sz)
                            g2 = mn_psp.tile([128, sz], F32,
                                             tag=f"g{lo}", bufs=1,
                                             name=f"g2_{s}_{lo}")
                            nc.tensor.matmul(
                                g2[:, :],
                                pts_sel5_sb[:, s * 128:(s + 1) * 128],
                                pts_own5_sb[:, sl],
                                start=True, stop=True)
                            nc.vector.tensor_scalar(
                                mn_sb[:, s, sl], g2[:, :], R2, MNEG,
                                OP.is_ge, OP.mult)

        # ================= layer 2 (per source slot) ====================
        with tc.tile_pool(name="prep2", bufs=2) as prep, \
             tc.tile_pool(name="prep_ps2", bufs=1, space="PSUM") as prep_ps:
            ht_own = prep.tile([128, 2, IC], BF16, tag="ht", bufs=1)
            for oc in range(2):
                ht_ps = prep_ps.tile([128, IC], F32, tag="ht_ps", bufs=1,
                                     name=f"ht_ps_2_{oc}")
                for s in range(2):
                    for lo, sz in ((0, 512), (512, 256)):
                        sl = slice(lo, lo + sz)
                        nc.tensor.matmul(
                            ht_ps[:, sl],
                            w2p_sb[:, s, oc * 128:(oc + 1) * 128],
                            xt_own[:, s, sl],
                            start=(s == 0), stop=(s == 1))
                nc.scalar.copy(ht_own[:, oc, :], ht_ps[:, :])

            edt_ps = prep_ps.tile([H, IC], F32, tag="edt", bufs=1)
            for s in range(2):
                for lo, sz in ((0, 512), (512, 256)):
                    sl = slice(lo, lo + sz)
                    nc.tensor.matmul(edt_ps[:, sl], adm2_sb[:, s, :],
                                     ht_own[:, s, sl],
                                     start=(s == 0), stop=(s == 1))
            nc.scalar.copy(edt_sb[:, :], edt_ps[:, :])
            for h in range(H):
                nc.sync.dma_start(out=edt_rows[0:1, h, :],
                                  in_=edt_sb[h:h + 1, :])
            for h in range(H):
                nc.gpsimd.partition_broadcast(ed_b[:, h, :],
                                              edt_rows[0:1, h, :])

        with tc.tile_pool(name="agg_ps2", bufs=1, space="PSUM") as agg_ps:
            aggp = [agg_ps.tile([128, 2, H, C], F32, tag=f"agg{p}",
                                name=f"agg_2_{p}")
                    for p in range(ICT // 2)]
            den_ps = agg_ps.tile([128, ICT, H], F32, tag="den",
                                 name="den_2")
            with tc.tile_pool(name="jl2", bufs=3) as jl, \
                 tc.tile_pool(name="h_ps2", bufs=2, space="PSUM") as h_psp:
                for s in range(NS):
                    # halo slots wait on the AllGather: tell the static
                    # scheduler to order their chain after the own slots
                    # so in-order engine queues don't stall behind it
                    if s >= OWN_SLOTS:
                        tc.tile_set_cur_wait(0.5)
                    # --- h + es for this slot's 128 sources ---
                    h_ps = h_psp.tile([128, HCE], F32, tag="h",
                                      name=f"h_ps_2_{s}")
                    if s < OWN_SLOTS:
                        for half in range(2):
                            nc.tensor.matmul(
                                h_ps[:, :],
                                xt_own[:, half, s * 128:(s + 1) * 128],
                                w2p_sb[:, half, :],
                                start=(half == 0), stop=(half == 1))
                    else:
                        xg = jl.tile([128, HC], BF16, tag="xg",
                                     name=f"xg_2_{s}")
                        nc.gpsimd.indirect_dma_start(
                            out=xg[:, :], out_offset=None,
                            in_=ag_out,
                            in_offset=bass.IndirectOffsetOnAxis(
                                ap=agidx_sb[:, s - OWN_SLOTS:
                                            s - OWN_SLOTS + 1],
                                axis=0))
                        xtg = jl.tile([128, 2, 128], BF16, tag="xtg",
                                      name=f"xtg_2_{s}")
                        for half in range(2):
                            t_ps = h_psp.tile(
                                [128, 128], BF16, tag=f"tr{half}",
                                bufs=1, name=f"tr_2_{s}_{half}")
                            nc.tensor.transpose(
                                t_ps[:, :],
                                xg[:, half * 128:(half + 1) * 128],
                                ident_bf[:, :])
                            nc.scalar.copy(xtg[:, half, :], t_ps[:, :])
                        for half in range(2):
                            nc.tensor.matmul(
                                h_ps[:, :], xtg[:, half, :],
                                w2p_sb[:, half, :],
                                start=(half == 0), stop=(half == 1))
                    nc.vector.tensor_scalar_add(es4[:, s, :],
                                                h_ps[:, HC:HCE], 0.0)
                    # h -> bf16 hi + lo with ones/zeros column
                    nc.scalar.copy(
                        hp_hi[:, s, :, 0:C],
                        h_ps[:, 0:HC].rearrange("p (h c) -> p h c", h=H))
                    nc.vector.tensor_tensor(
                        hp_lo[:, s, :, 0:C],
                        h_ps[:, 0:HC].rearrange("p (h c) -> p h c", h=H),
                        hp_hi[:, s, :, 0:C], OP.subtract)

                    # --- scores: L = leaky(ed+es) + mn ; A = exp(L) ---
                    L4 = jl.tile([128, H, IC], BF16, tag="L4",
                                 name=f"L4_2_{s}")
                    nc.scalar.activation(L4[:, 0, :], ed_b[:, 0, :],
                                         AF.Prelu,
                                         bias=es4[:, s, 0:1],
                                         scale=1.0, alpha=0.2)
                    T4 = jl.tile([128, 3, IC], BF16, tag="T4",
                                 name=f"T4_2_{s}")
                    for h in range(1, H):
                        nc.vector.tensor_scalar(
                            L4[:, h, :], ed_b[:, h, :],
                            es4[:, s, h:h + 1], None, OP.add)
                    nc.vector.tensor_scalar(
                        T4[:, :, :], L4[:, 1:4, :], 0.2, None, OP.mult)
                    nc.vector.tensor_tensor(L4[:, 1:4, :], L4[:, 1:4, :],
                                            T4[:, :, :], OP.max)
                    l4b, mnb = bass.broadcast_tensor_aps(
                        L4[:, :, :],
                        mn_sb[:, s:s + 1, :])
                    nc.vector.tensor_tensor(L4[:, :, :], l4b, mnb,
                                            OP.add)
                    A4 = jl.tile([128, H, IC], BF16, tag="A4",
                                 name=f"A4_2_{s}")
                    if s == NS - 1:
                        nc.scalar.activation(A4[:, 0:2, :], L4[:, 0:2, :],
                                             AF.Exp)
                        nc.scalar.activation(A4[:, 2:4, :], L4[:, 2:4, :],
                                             AF.Exp)
                    else:
                        nc.scalar.activation(A4[:, :, :], L4[:, :, :],
                                             AF.Exp)

                    # --- aggregation ---
                    for h in range(H):
                        for ic in range(ICT):
                            out_ap = aggp[ic // 2][:, ic % 2, h, :]
                            first = (s == 0 and h == 0 and ic % 2 == 0)
                            last = (s == NS - 1 and h == H - 1
                                    and ic % 2 == 1)
                            nc.tensor.matmul(
                                out_ap,
                                A4[:, h, ic * 128:(ic + 1) * 128],
                                hp_hi[:, s, h, 0:C],
                                start=first, stop=False)
                            nc.tensor.matmul(
                                out_ap,
                                A4[:, h, ic * 128:(ic + 1) * 128],
                                hp_lo[:, s, h, 0:C],
                                start=False, stop=last)
                            nc.tensor.matmul(
                                den_ps[:, ic, h:h + 1],
                                A4[:, h, ic * 128:(ic + 1) * 128],
                                hp_hi[:, s, h, C:CP1],
                                start=(s == 0 and h == 0 and ic == 0),
                                stop=(s == NS - 1 and h == H - 1
                                      and ic == ICT - 1))
                tc.cur_wait_ts = None

            # ---- finalize x2 = relu(num/den + b); fc ----
            with tc.tile_pool(name="fin2", bufs=2) as fin:
                for ic in range(ICT):
                    rec = fin.tile([128, H], F32, tag="rec",
                                   name=f"rec_2_{ic}")
                    nc.vector.reciprocal(rec[:, :], den_ps[:, ic, :])
                    for h in range(H):
                        nc.vector.scalar_tensor_tensor(
                            x_sb[:, ic, h * C:(h + 1) * C],
                            aggp[ic // 2][:, ic % 2, h, :],
                            rec[:, h:h + 1],
                            bias2_sb[:, h * C:(h + 1) * C],
                            OP.mult, OP.add)
                nc.vector.tensor_scalar(x_sb[:, :, :], x_sb[:, :, :],
                                        0.0, None, OP.max)
                for ic in range(ICT):
                    for o in range(2):
                        prod = fin.tile([128, HC], F32, tag="prod",
                                        name=f"prod_{ic}_{o}")
                        nc.vector.tensor_tensor(
                            prod[:, :], x_sb[:, ic, :],
                            fcw_sb[:, o * HC:(o + 1) * HC], OP.mult)
                        red = fin.tile([128, 1], F32, tag="red",
                                       name=f"red_{ic}_{o}")
                        nc.vector.tensor_reduce(
                            red[:, :], prod[:, :], AX.X, OP.add)
                        nc.vector.tensor_scalar_add(
                            logit_sb[:, ic, o:o + 1], red[:, :],
                            fcb_sb[:, o:o + 1])
                nc.sync.dma_start(
                    out=out_d.rearrange("(q p) o -> p q o", p=128),
                    in_=logit_sb[:, :, :])

    nc.compile()
    return nc


_BUILD_CACHE = {}


def _get_nc(nslot, bp, t1):
    key = (nslot, bp, t1)
    if key not in _BUILD_CACHE:
        _BUILD_CACHE[key] = build(nslot, bp, t1)
    return _BUILD_CACHE[key]


def _morton(p, bits=10):
    q = np.clip((p * (1 << bits)).astype(np.int64), 0, (1 << bits) - 1)
    code = np.zeros(len(p), np.int64)
    for b in range(bits):
        for dim in range(3):
            code |= ((q[:, dim] >> b) & 1) << (3 * b + dim)
    return code


def _plan(pts):
    """Sort nodes spatially, build compacted source lists for both layers."""
    order = np.argsort(_morton(pts), kind="stable")
    p_sorted = np.full((KP, 3), PAD_COORD, np.float32)
    p_sorted[:K] = pts[order]

    sq = (p_sorted ** 2).sum(-1, dtype=np.float32)
    G = p_sorted @ p_sorted.T
    d2 = sq[None, :] + sq[:, None] - 2.0 * G
    near = d2 < (R2 + MASK_EPS)          # [j, i], conservative superset

    halos = []
    for c in range(N_CORES):
        act = np.flatnonzero(near[:, c * IC:(c + 1) * IC].any(axis=1))
        halos.append(act[(act < c * IC) | (act >= (c + 1) * IC)])
    nslot = max(OWN_SLOTS + 1,
                max(OWN_SLOTS + (len(h) + 127) // 128 for h in halos))
    # permute each core's own nodes boundary-first so the AllGather only
    # ships the rows other cores actually fetch
    boundary = np.zeros(KP, bool)
    for h in halos:
        boundary[h] = True
    lists, agpos = [], np.zeros(KP, np.int64)
    bnd_counts = []
    for c in range(N_CORES):
        own = np.arange(c * IC, (c + 1) * IC)
        isb = boundary[own]
        perm = np.concatenate([own[isb], own[~isb]])
        bnd_counts.append(int(isb.sum()))
        lists.append(perm)
    bp = max(128, 128 * ((max(bnd_counts) + 127) // 128))
    for c in range(N_CORES):
        agpos[lists[c][:bnd_counts[c]]] = c * bp + np.arange(bnd_counts[c])
    # layer-1 per-chunk compacted source tiles (chunks in permuted order)
    raw1 = []
    for c in range(N_CORES):
        per_chunk = []
        for ci in range(ICT):
            dst = lists[c][ci * 128:(ci + 1) * 128]
            per_chunk.append(np.flatnonzero(near[:, dst].any(axis=1)))
        raw1.append(per_chunk)
    t1 = tuple(max(128, (max(len(raw1[c][ci]) for c in range(N_CORES))
                         + 127) // 128 * 128) // 128
               for ci in range(ICT))
    l1sels = []
    for c in range(N_CORES):
        parts = []
        for ci in range(ICT):
            src = raw1[c][ci]
            parts.append(np.concatenate(
                [src, np.full(t1[ci] * 128 - len(src), PAD_NODE,
                              src.dtype)]))
        l1sels.append(np.concatenate(parts))
    for c in range(N_CORES):
        l = np.concatenate([lists[c], halos[c]])
        lists[c] = np.concatenate(
            [l, np.full(nslot * 128 - len(l), PAD_NODE, l.dtype)])
    return order, p_sorted, lists, nslot, agpos, bp, l1sels, t1


def _prep_inputs(pos, pos_non_manifold, W1, a_src1, a_dst1, b1,
                 W2, a_src2, a_dst2, b2, fc_w, fc_b):
    bf16 = ml_dtypes.bfloat16
    pts = np.concatenate([np.asarray(pos, np.float32),
                          np.asarray(pos_non_manifold, np.float32)],
                         axis=2)[0].T  # [K, 3]
    order, p_sorted, lists, nslot, agpos, bp, l1sels, t1 = _plan(pts)
    sq_sorted = (p_sorted ** 2).sum(-1, dtype=np.float32).astype(np.float32)

    def bcast128(v):
        v = np.asarray(v, np.float32).reshape(-1)
        return np.ascontiguousarray(
            np.broadcast_to(v[None, :], (128, v.size)))

    def blockdiag(a):  # [H, C] -> [HC, H] fp32
        m = np.zeros((HC, H), dtype=np.float32)
        for h in range(H):
            m[h * C:(h + 1) * C, h] = np.asarray(a, np.float32)[h]
        return m

    def sel5_of(sel):
        psel = p_sorted[sel]
        return np.ascontiguousarray(np.concatenate(
            [psel.T, sq_sorted[sel][None, :],
             np.ones((1, len(sel)), np.float32)], axis=0)
            .astype(np.float32))

    W1f = np.asarray(W1, np.float32)
    W2f = np.asarray(W2, np.float32)
    w1p = np.concatenate([W1f, W1f @ blockdiag(a_src1)], axis=1)
    w2p = np.concatenate([W2f, W2f @ blockdiag(a_src2)], axis=1)

    shared = {
        "w1p": np.ascontiguousarray(w1p.astype(np.float32)),
        "w2p": np.ascontiguousarray(w2p.astype(bf16)),
        "adm1": blockdiag(a_dst1).astype(bf16),
        "adm2": blockdiag(a_dst2).astype(bf16),
        "bias1": bcast128(b1),
        "bias2": bcast128(b2),
        "fcw": bcast128(np.asarray(fc_w, np.float32).T),
        "fcb": bcast128(fc_b),
        "ident": np.eye(128, dtype=np.float32),
    }
    in_maps = []
    for c in range(N_CORES):
        sel = lists[c]
        pown = p_sorted[sel[:IC]]                 # own nodes, boundary-first
        own5 = np.concatenate(
            [-2.0 * pown.T, np.ones((1, IC), np.float32),
             (pown ** 2).sum(-1, dtype=np.float32)[None, :]], axis=0)
        # halo slots index boundary-layout ag rows via the host map
        agidx = np.ascontiguousarray(
            agpos[sel[OWN_SLOTS * 128:]].reshape(-1, 128).T
            .astype(np.int32))
        m = dict(shared)
        m["pts_sel5"] = sel5_of(sel)
        m["pts1"] = sel5_of(l1sels[c])
        m["pts_own5"] = np.ascontiguousarray(own5.astype(np.float32))
        m["pts_own3"] = np.ascontiguousarray(pown.T)
        m["agidx"] = agidx
        in_maps.append(m)
    return in_maps, order, nslot, lists, bp, t1


def kernel(pos, pos_non_manifold, W1, a_src1, a_dst1, b1,
           W2, a_src2, a_dst2, b2, fc_w, fc_b, _trace=False):
    in_maps, order, nslot, lists, bp, t1 = _prep_inputs(
        pos, pos_non_manifold, W1, a_src1, a_dst1, b1,
        W2, a_src2, a_dst2, b2, fc_w, fc_b)
    nc = _get_nc(nslot, bp, t1)
    res = run_bass_kernel_spmd(nc, in_maps, core_ids=list(range(N_CORES)),
                               trace=_trace)
    kernel.last_results = res
    x2s = np.concatenate([res.results[c]["out"] for c in range(N_CORES)],
                         axis=0)  # [KP, 2], rows in per-core list order
    perm = np.concatenate([lists[c][:IC] for c in range(N_CORES)])
    x2p = np.empty((KP, 2), np.float32)
    x2p[perm] = x2s
    x2 = np.empty((K, 2), np.float32)
    x2[order] = x2p[:K]
    logits = np.ascontiguousarray(x2[M:K]).reshape(1, 2, 3000)
    return logits.astype(np.float32)


# revision 68
# speedup vs baseline: 1.0538x; 1.0538x over previous
"""Trainium2 Bass kernel for a 2-layer GAT occupancy predictor (B=1).

Reference math:
  pts = concat(pos, pos_non_manifold) -> [K=6000, 3]
  mask[i,j] = ||pts_i - pts_j||^2 < 0.05^2          (dense radius graph)
  layer l:  h = x @ Wl                              [K, 4*64]
            e[i,j,h] = leaky02(ed[i,h] + es[j,h])   es/ed = <h, a_src/dst>
            alpha = softmax_j(e masked)
            x' = relu(alpha @ h + b)
  logits = (x2 @ fc_w + fc_b)[M:] reshaped to [1, 2, 3000]

Distribution (8 NeuronCores): nodes are Morton-sorted on the host so the
radius graph becomes block-local; core c owns destination rows
[768c, 768(c+1)) of the sorted, padded 6144-node graph.

Layer 1 is fully static and runs per destination chunk: for each of the 6
own 128-node chunks the host compacts the ~200 in-radius source nodes into
2-3 tiles (padded per chunk position across cores), and the score pipeline
runs merged over the chunk's tiles and all 4 heads via stride-0 broadcast
APs (one add, one 0.2x, one max, one mask-add, one exp per chunk).

Layer 2 runs per whole-core source slot: the ~870 unique sources are
compacted into 8 slots of 128 ordered [own 768 | halo | pad].  Each core's
own nodes are permuted boundary-first so the single bf16 AllGather between
layers only ships the rows other cores fetch; layer 2 computes own-slot h
from the local transposed features and fetches the halo slots' x rows with
one indirect DMA per slot, transposing on the PE.  The halo chain is
pushed late in the static schedule (tile_set_cur_wait) so the in-order
engine queues don't stall on the collective.  Layer-2 slot masks are
computed from static coordinates during the inter-layer window, filling
the AllGather bubble.

Engine mapping:
  PE   : h (with es riding along as 4 extra host-folded weight columns),
         d2 = |p_i - p_j|^2 as K=5 matmuls ([p; sq; 1] x [-2p; 1; sq]),
         the alpha @ h aggregation as bf16 hi+lo pairs into shared PSUM
         banks, denominators as N=1 ones-column matmuls.
  DVE  : scores, leaky, mask-adds (broadcast tensor_tensor), h hi/lo
         split, finalize division/relu, fc.
  ACT  : layer-2 head-0 leaky via Prelu, exp, PSUM->SBUF copies.
  Pool : partition-broadcast of ed, indirect halo gathers, AllGather.
Mask offsets (-30/0 bf16): per-pair tiles in layer 1, SBUF-resident
per-slot planes for layer 2.
Padded nodes sit at (-1,-1,-1): finite features, outside every real radius.
"""

import sys

sys.path.insert(0, "/opt/trn_rl_repo")

from contextlib import ExitStack

import ml_dtypes
import numpy as np

import concourse.bacc as bacc
import concourse.bass as bass
import concourse.mybir as mybir
import concourse.tile as tile
from concourse.bass_utils import run_bass_kernel_spmd

F32 = mybir.dt.float32
BF16 = mybir.dt.bfloat16
I32 = mybir.dt.int32
AF = mybir.ActivationFunctionType
OP = mybir.AluOpType
AX = mybir.AxisListType

N_CORES = 8
N = 3000
M = 3000
K = N + M          # real nodes
KP = 6144          # padded nodes
NT = KP // 128     # 48
IC = KP // N_CORES # 768 destinations per core
ICT = IC // 128    # 6 destination chunks per core
OWN_SLOTS = ICT    # first 6 layer-2 slots are the core's own nodes
H = 4              # heads
C = 64             # channels per head
HC = H * C         # 256
HCE = HC + H       # h columns + es columns
CP1 = C + 1        # head channels + ones column
R2 = float(np.float32(0.05) * np.float32(0.05))
PAD_COORD = -1.0
MASK_EPS = 1e-5    # host activity-test margin (superset of device mask)
MNEG = -30.0       # masked-score offset: exp(-30+L) ~ 1e-12
PAD_NODE = KP - 1  # all-padding node, used for unused slot entries


def build(nslot, bp, t1, n_cores=N_CORES, fake_ag=False):
    nc = bacc.Bacc("TRN2", target_bir_lowering=False, debug=False,
                   num_devices=n_cores)
    NS = nslot
    NH = NS - OWN_SLOTS          # halo slots (gathered in layer 2)
    BP = bp                      # boundary rows shipped in the AllGather
    NT1 = sum(t1)                # layer-1 chunk-tile count
    T1MAX = max(t1)
    assert NH >= 1 and BP % 128 == 0 and len(t1) == ICT

    # ---- kernel I/O (identical program on every core) ----
    # *sel5 rows: [p(3); sq; ones] for sources
    # pts_own5 rows: [-2p(3); ones; sq] for the own destination columns
    pts_sel5_d = nc.dram_tensor("pts_sel5", [5, NS * 128], F32,
                                kind="ExternalInput")
    pts1_d = nc.dram_tensor("pts1", [5, NT1 * 128], F32,
                            kind="ExternalInput")
    pts_own5_d = nc.dram_tensor("pts_own5", [5, IC], F32,
                                kind="ExternalInput")
    pts_own3_d = nc.dram_tensor("pts_own3", [3, IC], F32,
                                kind="ExternalInput")
    agidx_d = nc.dram_tensor("agidx", [128, NH], I32, kind="ExternalInput")
    # w1p/w2p: [W | W @ a_src_blockdiag] so es rides along with h
    w1p_d = nc.dram_tensor("w1p", [3, HCE], F32, kind="ExternalInput")
    w2p_d = nc.dram_tensor("w2p", [HC, HCE], BF16, kind="ExternalInput")
    adm1_d = nc.dram_tensor("adm1", [HC, H], BF16, kind="ExternalInput")
    adm2_d = nc.dram_tensor("adm2", [HC, H], BF16, kind="ExternalInput")
    bias1_d = nc.dram_tensor("bias1", [128, HC], F32, kind="ExternalInput")
    bias2_d = nc.dram_tensor("bias2", [128, HC], F32, kind="ExternalInput")
    fcw_d = nc.dram_tensor("fcw", [128, 2 * HC], F32, kind="ExternalInput")
    fcb_d = nc.dram_tensor("fcb", [128, 2], F32, kind="ExternalInput")
    ident_d = nc.dram_tensor("ident", [128, 128], F32, kind="ExternalInput")

    out_d = nc.dram_tensor("out", [IC, 2], F32, kind="ExternalOutput")

    with tile.TileContext(nc) as tc, ExitStack() as st:
        dram = st.enter_context(tc.tile_pool(name="dram", bufs=1,
                                             space="DRAM"))
        x_bounce = dram.tile([BP, HC], BF16)
        ag_out = dram.tile([n_cores * BP, HC], BF16,
                           addr_space=("Local" if fake_ag else "Shared"))

        const = st.enter_context(tc.tile_pool(name="const", bufs=1))
        pts_sel5_sb = const.tile([5, NS * 128], F32)
        pts1_sb = const.tile([5, NT1 * 128], F32)
        pts_own5_sb = const.tile([5, IC], F32)
        pts_own3_sb = const.tile([3, IC], F32)
        agidx_sb = const.tile([128, NH], I32)
        w1p_sb = const.tile([3, HCE], F32)
        w2p_sb = const.tile([128, 2, HCE], BF16)
        adm1_sb = const.tile([128, 2, H], BF16)
        adm2_sb = const.tile([128, 2, H], BF16)
        bias1_sb = const.tile([128, HC], F32)
        bias2_sb = const.tile([128, HC], F32)
        fcw_sb = const.tile([128, 2 * HC], F32)
        fcb_sb = const.tile([128, 2], F32)
        ident_sb = const.tile([128, 128], F32)
        ident_bf = const.tile([128, 128], BF16)
        w1b = const.tile([3, HCE], BF16)
        own3b = const.tile([3, IC], BF16)

        nc.sync.dma_start(out=pts_sel5_sb[:, :], in_=pts_sel5_d[:, :])
        nc.sync.dma_start(out=pts1_sb[:, :], in_=pts1_d[:, :])
        nc.sync.dma_start(out=pts_own5_sb[:, :], in_=pts_own5_d[:, :])
        nc.sync.dma_start(out=pts_own3_sb[:, :], in_=pts_own3_d[:, :])
        nc.sync.dma_start(out=agidx_sb[:, :], in_=agidx_d[:, :])
        nc.sync.dma_start(out=w1p_sb[:, :], in_=w1p_d[:, :])
        nc.sync.dma_start(out=w2p_sb[:, :, :],
                          in_=w2p_d.rearrange("(s p) c -> p s c", p=128))
        nc.sync.dma_start(out=adm1_sb[:, :, :],
                          in_=adm1_d.rearrange("(s p) h -> p s h", p=128))
        nc.sync.dma_start(out=adm2_sb[:, :, :],
                          in_=adm2_d.rearrange("(s p) h -> p s h", p=128))
        nc.sync.dma_start(out=bias1_sb[:, :], in_=bias1_d[:, :])
        nc.sync.dma_start(out=bias2_sb[:, :], in_=bias2_d[:, :])
        nc.sync.dma_start(out=fcw_sb[:, :], in_=fcw_d[:, :])
        nc.sync.dma_start(out=fcb_sb[:, :], in_=fcb_d[:, :])
        nc.sync.dma_start(out=ident_sb[:, :], in_=ident_d[:, :])
        nc.scalar.copy(ident_bf[:, :], ident_sb[:, :])
        nc.scalar.copy(w1b[:, :], w1p_sb[:, :])
        nc.scalar.copy(own3b[:, :], pts_own3_sb[:, :])

        big = st.enter_context(tc.tile_pool(name="big", bufs=1))
        hp1_hi = big.tile([128, NT1, H, CP1], BF16)
        hp1_lo = big.tile([128, NT1, H, CP1], BF16)
        es41 = big.tile([128, NT1, H], BF16)
        hp_hi = big.tile([128, NS, H, CP1], BF16)
        hp_lo = big.tile([128, NS, H, CP1], BF16)
        es4 = big.tile([128, NS, H], F32)
        ed_b = big.tile([128, H, IC], BF16)
        x_sb = big.tile([128, ICT, HC], F32)
        x_bf = big.tile([128, ICT, HC], BF16)
        xt_own = big.tile([128, 2, IC], BF16)
        edt_sb = big.tile([H, IC], BF16)
        edt_rows = big.tile([1, H, IC], BF16)
        logit_sb = big.tile([128, ICT, 2], F32)
        mn_sb = big.tile([128, NS, IC], BF16)   # layer-2 slot mask offsets

        nc.vector.memset(hp1_hi[:, :, :, C:CP1], 1.0)
        nc.vector.memset(hp1_lo[:, :, :, C:CP1], 0.0)
        nc.vector.memset(hp_hi[:, :, :, C:CP1], 1.0)
        nc.vector.memset(hp_lo[:, :, :, C:CP1], 0.0)

        # ================= layer 1 (per destination chunk) ==============
        # ---- own-column side: hT(own), edT, ED broadcasts ----
        with tc.tile_pool(name="prep1", bufs=2) as prep, \
             tc.tile_pool(name="prep_ps1", bufs=1, space="PSUM") as prep_ps:
            ht_own = prep.tile([128, 2, IC], BF16, tag="ht", bufs=1)
            for oc in range(2):
                ht_ps = prep_ps.tile([128, IC], F32, tag="ht_ps", bufs=1,
                                     name=f"ht_ps_1_{oc}")
                for lo, sz in ((0, 512), (512, 256)):
                    sl = slice(lo, lo + sz)
                    nc.tensor.matmul(
                        ht_ps[:, sl], w1b[:, oc * 128:(oc + 1) * 128],
                        own3b[:, sl], start=True, stop=True)
                nc.scalar.copy(ht_own[:, oc, :], ht_ps[:, :])

            edt_ps = prep_ps.tile([H, IC], F32, tag="edt", bufs=1)
            for s in range(2):
                for lo, sz in ((0, 512), (512, 256)):
                    sl = slice(lo, lo + sz)
                    nc.tensor.matmul(edt_ps[:, sl], adm1_sb[:, s, :],
                                     ht_own[:, s, sl],
                                     start=(s == 0), stop=(s == 1))
            nc.scalar.copy(edt_sb[:, :], edt_ps[:, :])
            for h in range(H):
                nc.sync.dma_start(out=edt_rows[0:1, h, :],
                                  in_=edt_sb[h:h + 1, :])
            # stagger: first chunks' columns land before the full rows so
            # the chunk loop can start scoring sooner
            for h in range(H):
                nc.gpsimd.partition_broadcast(ed_b[:, h, 0:256],
                                              edt_rows[0:1, h, 0:256])
            for h in range(H):
                nc.gpsimd.partition_broadcast(ed_b[:, h, 256:IC],
                                              edt_rows[0:1, h, 256:IC])

        # ---- chunk loop: h+es, per-pair masks, merged scores, agg ----
        with tc.tile_pool(name="agg_ps1", bufs=1, space="PSUM") as agg_ps:
            aggp = [agg_ps.tile([128, 2, H, C], F32, tag=f"agg{p}",
                                name=f"agg_1_{p}")
                    for p in range(ICT // 2)]
            den_ps = agg_ps.tile([128, ICT, H], F32, tag="den",
                                 name="den_1")
            with tc.tile_pool(name="jl1", bufs=3) as jl, \
                 tc.tile_pool(name="h_ps1", bufs=2, space="PSUM") as h_psp:
                # pass 1: per-tile h/es/hi-lo/masks (independent of ed_b,
                # fills the DVE queue while the prep broadcasts run)
                mnCs = []
                tbase = 0
                for ci in range(ICT):
                    Tc = t1[ci]
                    csl = slice(ci * 128, (ci + 1) * 128)
                    mnC = jl.tile([128, T1MAX, 128], BF16, tag="mn",
                                  bufs=ICT, name=f"mn1_{ci}")
                    mnCs.append(mnC)
                    for t in range(Tc):
                        s = tbase + t
                        ssl = slice(s * 128, (s + 1) * 128)
                        h_ps = h_psp.tile([128, HCE], F32, tag="h",
                                          name=f"h1_{s}")
                        nc.tensor.matmul(h_ps[:, :], pts1_sb[0:3, ssl],
                                         w1p_sb[:, :],
                                         start=True, stop=True)
                        nc.vector.tensor_scalar_add(
                            es41[:, s, :], h_ps[:, HC:HCE], 0.0)
                        nc.scalar.copy(
                            hp1_hi[:, s, :, 0:C],
                            h_ps[:, 0:HC].rearrange("p (h c) -> p h c",
                                                    h=H))
                        nc.vector.tensor_tensor(
                            hp1_lo[:, s, :, 0:C],
                            h_ps[:, 0:HC].rearrange("p (h c) -> p h c",
                                                    h=H),
                            hp1_hi[:, s, :, 0:C], OP.subtract)
                        g_ps = h_psp.tile([128, 128], F32, tag="g",
                                          bufs=2, name=f"g1_{s}")
                        nc.tensor.matmul(g_ps[:, :], pts1_sb[:, ssl],
                                         pts_own5_sb[:, csl],
                                         start=True, stop=True)
                        nc.vector.tensor_scalar(
                            mnC[:, t, :], g_ps[:, :], R2, MNEG,
                            OP.is_ge, OP.mult)
                    tbase += Tc

                # pass 2: merged scores + aggregation per chunk
                tbase = 0
                for ci in range(ICT):
                    Tc = t1[ci]
                    csl = slice(ci * 128, (ci + 1) * 128)
                    mnC = mnCs[ci]
                    L = jl.tile([128, Tc, H, 128], BF16, tag="L4",
                                name=f"L1_{ci}")
                    e1 = ed_b[:, :, csl].rearrange(
                        "p (o h) d -> p o h d", o=1)
                    e2 = es41[:, tbase:tbase + Tc, :].rearrange(
                        "p t (h o) -> p t h o", o=1)
                    b1, b2 = bass.broadcast_tensor_aps(e1, e2)
                    nc.vector.tensor_tensor(L[:, :, :, :], b1, b2, OP.add)
                    T4 = jl.tile([128, Tc, H, 128], BF16, tag="T4",
                                 name=f"T4_1_{ci}")
                    nc.vector.tensor_scalar(T4[:, :, :, :], L[:, :, :, :],
                                            0.2, None, OP.mult)
                    nc.vector.tensor_tensor(L[:, :, :, :], L[:, :, :, :],
                                            T4[:, :, :, :], OP.max)
                    mb = mnC[:, 0:Tc, :].rearrange(
                        "p t (o d) -> p t o d", o=1)
                    bl, bm = bass.broadcast_tensor_aps(L[:, :, :, :], mb)
                    nc.vector.tensor_tensor(L[:, :, :, :], bl, bm, OP.add)
                    A = jl.tile([128, Tc, H, 128], BF16, tag="A4",
                                name=f"A1_{ci}")
                    if ci == ICT - 1:
                        nc.scalar.activation(A[:, 0:1, :, :],
                                             L[:, 0:1, :, :], AF.Exp)
                        nc.scalar.activation(A[:, 1:Tc, :, :],
                                             L[:, 1:Tc, :, :], AF.Exp)
                    else:
                        nc.scalar.activation(A[:, :, :, :], L[:, :, :, :],
                                             AF.Exp)

                    bank = aggp[ci // 2]
                    half = ci % 2
                    for t in range(Tc):
                        s = tbase + t
                        for h in range(H):
                            first = (half == 0 and t == 0 and h == 0)
                            last = (half == 1 and t == Tc - 1
                                    and h == H - 1)
                            nc.tensor.matmul(
                                bank[:, half, h, :], A[:, t, h, :],
                                hp1_hi[:, s, h, 0:C],
                                start=first, stop=False)
                            nc.tensor.matmul(
                                bank[:, half, h, :], A[:, t, h, :],
                                hp1_lo[:, s, h, 0:C],
                                start=False, stop=last)
                            nc.tensor.matmul(
                                den_ps[:, ci, h:h + 1], A[:, t, h, :],
                                hp1_hi[:, s, h, C:CP1],
                                start=(ci == 0 and t == 0 and h == 0),
                                stop=(ci == ICT - 1 and t == Tc - 1
                                      and h == H - 1))
                    tbase += Tc

            # ---- finalize x1, AllGather, transposes, layer-2 masks ----
            with tc.tile_pool(name="fin1", bufs=2) as fin, \
                 tc.tile_pool(name="fin_ps1", bufs=2,
                              space="PSUM") as fin_ps:
                qb = BP // 128
                for ic in range(ICT):
                    rec = fin.tile([128, H], F32, tag="rec",
                                   name=f"rec_1_{ic}")
                    nc.vector.reciprocal(rec[:, :], den_ps[:, ic, :])
                    for h in range(H):
                        nc.vector.scalar_tensor_tensor(
                            x_sb[:, ic, h * C:(h + 1) * C],
                            aggp[ic // 2][:, ic % 2, h, :],
                            rec[:, h:h + 1],
                            bias1_sb[:, h * C:(h + 1) * C],
                            OP.mult, OP.add)
                    if ic == qb - 1:
                        # boundary chunks done: fire the AllGather now,
                        # before the rest of the finalize
                        nc.vector.tensor_scalar(
                            x_sb[:, 0:qb, :], x_sb[:, 0:qb, :],
                            0.0, None, OP.max)
                        nc.scalar.copy(x_bf[:, 0:qb, :],
                                       x_sb[:, 0:qb, :])
                        nc.sync.dma_start(
                            out=x_bounce.rearrange("(q p) c -> p q c",
                                                   p=128),
                            in_=x_bf[:, 0:qb, :])
                        if fake_ag:
                            for r in range(n_cores):
                                nc.sync.dma_start(
                                    out=ag_out[r * BP:(r + 1) * BP, :],
                                    in_=x_bounce[:, :])
                        else:
                            nc.gpsimd.collective_compute(
                                "AllGather", OP.bypass,
                                replica_groups=[list(range(n_cores))],
                                ins=[x_bounce.opt()],
                                outs=[ag_out.opt()])
                nc.vector.tensor_scalar(x_sb[:, qb:, :], x_sb[:, qb:, :],
                                        0.0, None, OP.max)
                # transposed own x, needed for layer-2 prep
                for ic in range(ICT):
                    for oc in range(2):
                        t_ps = fin_ps.tile([128, 128], F32, tag="t_ps",
                                           name=f"t_ps_{ic}_{oc}")
                        nc.tensor.transpose(
                            t_ps[:, :],
                            x_sb[:, ic, oc * 128:(oc + 1) * 128],
                            ident_sb[:, :])
                        nc.scalar.copy(
                            xt_own[:, oc, ic * 128:(ic + 1) * 128],
                            t_ps[:, :])
                # layer-2 slot masks from static coords: fills the
                # AllGather bubble
                with tc.tile_pool(name="mn_ps2", bufs=1,
                                  space="PSUM") as mn_psp:
                    for s in range(NS):
                        for lo, sz in ((0, 512), (512, 256)):
                            sl = slice(lo, lo + sz)
                            g2 = mn_psp.tile([128, sz], F32,
                                             tag=f"g{lo}", bufs=1,
                                             name=f"g2_{s}_{lo}")
                            nc.tensor.matmul(
                                g2[:, :],
                                pts_sel5_sb[:, s * 128:(s + 1) * 128],
                                pts_own5_sb[:, sl],
                                start=True, stop=True)
                            nc.vector.tensor_scalar(
                                mn_sb[:, s, sl], g2[:, :], R2, MNEG,
                                OP.is_ge, OP.mult)

        # ================= layer 2 (per source slot) ====================
        with tc.tile_pool(name="prep2", bufs=2) as prep, \
             tc.tile_pool(name="prep_ps2", bufs=1, space="PSUM") as prep_ps:
            ht_own = prep.tile([128, 2, IC], BF16, tag="ht", bufs=1)
            for oc in range(2):
                ht_ps = prep_ps.tile([128, IC], F32, tag="ht_ps", bufs=1,
                                     name=f"ht_ps_2_{oc}")
                for s in range(2):
                    for lo, sz in ((0, 512), (512, 256)):
                        sl = slice(lo, lo + sz)
                        nc.tensor.matmul(
                            ht_ps[:, sl],
                            w2p_sb[:, s, oc * 128:(oc + 1) * 128],
                            xt_own[:, s, sl],
                            start=(s == 0), stop=(s == 1))
                nc.scalar.copy(ht_own[:, oc, :], ht_ps[:, :])

            edt_ps = prep_ps.tile([H, IC], F32, tag="edt", bufs=1)
            for s in range(2):
                for lo, sz in ((0, 512), (512, 256)):
                    sl = slice(lo, lo + sz)
                    nc.tensor.matmul(edt_ps[:, sl], adm2_sb[:, s, :],
                                     ht_own[:, s, sl],
                                     start=(s == 0), stop=(s == 1))
            nc.scalar.copy(edt_sb[:, :], edt_ps[:, :])
            for h in range(H):
                nc.sync.dma_start(out=edt_rows[0:1, h, :],
                                  in_=edt_sb[h:h + 1, :])
            for h in range(H):
                nc.gpsimd.partition_broadcast(ed_b[:, h, :],
                                              edt_rows[0:1, h, :])

        with tc.tile_pool(name="agg_ps2", bufs=1, space="PSUM") as agg_ps:
            aggp = [agg_ps.tile([128, 2, H, C], F32, tag=f"agg{p}",
                                name=f"agg_2_{p}")
                    for p in range(ICT // 2)]
            den_ps = agg_ps.tile([128, ICT, H], F32, tag="den",
                                 name="den_2")
            with tc.tile_pool(name="jl2", bufs=3) as jl, \
                 tc.tile_pool(name="h_ps2", bufs=2, space="PSUM") as h_psp:
                for s in range(NS):
                    # halo slots wait on the AllGather: tell the static
                    # scheduler to order their chain after the own slots
                    # so in-order engine queues don't stall behind it
                    if s >= OWN_SLOTS:
                        tc.tile_set_cur_wait(0.5)
                    # --- h + es for this slot's 128 sources ---
                    h_ps = h_psp.tile([128, HCE], F32, tag="h",
                                      name=f"h_ps_2_{s}")
                    if s < OWN_SLOTS:
                        for half in range(2):
                            nc.tensor.matmul(
                                h_ps[:, :],
                                xt_own[:, half, s * 128:(s + 1) * 128],
                                w2p_sb[:, half, :],
                                start=(half == 0), stop=(half == 1))
                    else:
                        xg = jl.tile([128, HC], BF16, tag="xg",
                                     name=f"xg_2_{s}")
                        nc.gpsimd.indirect_dma_start(
                            out=xg[:, :], out_offset=None,
                            in_=ag_out,
                            in_offset=bass.IndirectOffsetOnAxis(
                                ap=agidx_sb[:, s - OWN_SLOTS:
                                            s - OWN_SLOTS + 1],
                                axis=0))
                        xtg = jl.tile([128, 2, 128], BF16, tag="xtg",
                                      name=f"xtg_2_{s}")
                        for half in range(2):
                            t_ps = h_psp.tile(
                                [128, 128], BF16, tag=f"tr{half}",
                                bufs=1, name=f"tr_2_{s}_{half}")
                            nc.tensor.transpose(
                                t_ps[:, :],
                                xg[:, half * 128:(half + 1) * 128],
                                ident_bf[:, :])
                            nc.scalar.copy(xtg[:, half, :], t_ps[:, :])
                        for half in range(2):
                            nc.tensor.matmul(
                                h_ps[:, :], xtg[:, half, :],
                                w2p_sb[:, half, :],
                                start=(half == 0), stop=(half == 1))
                    nc.vector.tensor_scalar_add(es4[:, s, :],
                                                h_ps[:, HC:HCE], 0.0)
                    # h -> bf16 hi + lo with ones/zeros column
                    nc.scalar.copy(
                        hp_hi[:, s, :, 0:C],
                        h_ps[:, 0:HC].rearrange("p (h c) -> p h c", h=H))
                    nc.vector.tensor_tensor(
                        hp_lo[:, s, :, 0:C],
                        h_ps[:, 0:HC].rearrange("p (h c) -> p h c", h=H),
                        hp_hi[:, s, :, 0:C], OP.subtract)

                    # --- scores: L = leaky(ed+es) + mn ; A = exp(L) ---
                    L4 = jl.tile([128, H, IC], BF16, tag="L4",
                                 name=f"L4_2_{s}")
                    nc.scalar.activation(L4[:, 0, :], ed_b[:, 0, :],
                                         AF.Prelu,
                                         bias=es4[:, s, 0:1],
                                         scale=1.0, alpha=0.2)
                    T4 = jl.tile([128, 3, IC], BF16, tag="T4",
                                 name=f"T4_2_{s}")
                    for h in range(1, H):
                        nc.vector.tensor_scalar(
                            L4[:, h, :], ed_b[:, h, :],
                            es4[:, s, h:h + 1], None, OP.add)
                    nc.vector.tensor_scalar(
                        T4[:, :, :], L4[:, 1:4, :], 0.2, None, OP.mult)
                    nc.vector.tensor_tensor(L4[:, 1:4, :], L4[:, 1:4, :],
                                            T4[:, :, :], OP.max)
                    l4b, mnb = bass.broadcast_tensor_aps(
                        L4[:, :, :],
                        mn_sb[:, s:s + 1, :])
                    nc.vector.tensor_tensor(L4[:, :, :], l4b, mnb,
                                            OP.add)
                    A4 = jl.tile([128, H, IC], BF16, tag="A4",
                                 name=f"A4_2_{s}")
                    if s == NS - 1:
                        nc.scalar.activation(A4[:, 0:2, :], L4[:, 0:2, :],
                                             AF.Exp)
                        nc.scalar.activation(A4[:, 2:4, :], L4[:, 2:4, :],
                                             AF.Exp)
                    else:
                        nc.scalar.activation(A4[:, :, :], L4[:, :, :],
                                             AF.Exp)

                    # --- aggregation ---
                    for h in range(H):
                        for ic in range(ICT):
                            out_ap = aggp[ic // 2][:, ic % 2, h, :]
                            first = (s == 0 and h == 0 and ic % 2 == 0)
                            last = (s == NS - 1 and h == H - 1
                                    and ic % 2 == 1)
                            nc.tensor.matmul(
                                out_ap,
                                A4[:, h, ic * 128:(ic + 1) * 128],
                                hp_hi[:, s, h, 0:C],
                                start=first, stop=False)
                            nc.tensor.matmul(
                                out_ap,
                                A4[:, h, ic * 128:(ic + 1) * 128],
                                hp_lo[:, s, h, 0:C],
                                start=False, stop=last)
                            nc.tensor.matmul(
                                den_ps[:, ic, h:h + 1],
                                A4[:, h, ic * 128:(ic + 1) * 128],
                                hp_hi[:, s, h, C:CP1],
                                start=(s == 0 and h == 0 and ic == 0),
                                stop=(s == NS - 1 and h == H - 1
                                      and ic == ICT - 1))
                tc.cur_wait_ts = None

            # ---- finalize x2 = relu(num/den + b); fc ----
            with tc.tile_pool(name="fin2", bufs=2) as fin:
                for ic in range(ICT):
                    rec = fin.tile([128, H], F32, tag="rec",
                                   name=f"rec_2_{ic}")
                    nc.vector.reciprocal(rec[:, :], den_ps[:, ic, :])
                    for h in range(H):
                        nc.vector.scalar_tensor_tensor(
                            x_sb[:, ic, h * C:(h + 1) * C],
                            aggp[ic // 2][:, ic % 2, h, :],
                            rec[:, h:h + 1],
                            bias2_sb[:, h * C:(h + 1) * C],
                            OP.mult, OP.add)
                nc.vector.tensor_scalar(x_sb[:, :, :], x_sb[:, :, :],
                                        0.0, None, OP.max)
                for ic in range(ICT):
                    for o in range(2):
                        prod = fin.tile([128, HC], F32, tag="prod",
                                        name=f"prod_{ic}_{o}")
                        nc.vector.tensor_tensor(
                            prod[:, :], x_sb[:, ic, :],
                            fcw_sb[:, o * HC:(o + 1) * HC], OP.mult)
                        red = fin.tile([128, 1], F32, tag="red",
                                       name=f"red_{ic}_{o}")
                        nc.vector.tensor_reduce(
                            red[:, :], prod[:, :], AX.X, OP.add)
                        nc.vector.tensor_scalar_add(
                            logit_sb[:, ic, o:o + 1], red[:, :],
                            fcb_sb[:, o:o + 1])
                nc.sync.dma_start(
                    out=out_d.rearrange("(q p) o -> p q o", p=128),
                    in_=logit_sb[:, :, :])

    nc.compile()
    return nc


_BUILD_CACHE = {}


def _get_nc(nslot, bp, t1):
    key = (nslot, bp, t1)
    if key not in _BUILD_CACHE:
        _BUILD_CACHE[key] = build(nslot, bp, t1)
    return _BUILD_CACHE[key]


def _morton(p, bits=10):
    q = np.clip((p * (1 << bits)).astype(np.int64), 0, (1 << bits) - 1)
    code = np.zeros(len(p), np.int64)
    for b in range(bits):
        for dim in range(3):
            code |= ((q[:, dim] >> b) & 1) << (3 * b + dim)
    return code


def _plan(pts):
    """Sort nodes spatially, build compacted source lists for both layers."""
    order = np.argsort(_morton(pts), kind="stable")
    p_sorted = np.full((KP, 3), PAD_COORD, np.float32)
    p_sorted[:K] = pts[order]

    sq = (p_sorted ** 2).sum(-1, dtype=np.float32)
    G = p_sorted @ p_sorted.T
    d2 = sq[None, :] + sq[:, None] - 2.0 * G
    near = d2 < (R2 + MASK_EPS)          # [j, i], conservative superset

    halos = []
    for c in range(N_CORES):
        act = np.flatnonzero(near[:, c * IC:(c + 1) * IC].any(axis=1))
        halos.append(act[(act < c * IC) | (act >= (c + 1) * IC)])
    nslot = max(OWN_SLOTS + 1,
                max(OWN_SLOTS + (len(h) + 127) // 128 for h in halos))
    # permute each core's own nodes boundary-first so the AllGather only
    # ships the rows other cores actually fetch
    boundary = np.zeros(KP, bool)
    for h in halos:
        boundary[h] = True
    lists, agpos = [], np.zeros(KP, np.int64)
    bnd_counts = []
    for c in range(N_CORES):
        own = np.arange(c * IC, (c + 1) * IC)
        isb = boundary[own]
        perm = np.concatenate([own[isb], own[~isb]])
        bnd_counts.append(int(isb.sum()))
        lists.append(perm)
    bp = max(128, 128 * ((max(bnd_counts) + 127) // 128))
    for c in range(N_CORES):
        agpos[lists[c][:bnd_counts[c]]] = c * bp + np.arange(bnd_counts[c])
    # layer-1 per-chunk compacted source tiles (chunks in permuted order)
    raw1 = []
    for c in range(N_CORES):
        per_chunk = []
        for ci in range(ICT):
            dst = lists[c][ci * 128:(ci + 1) * 128]
            per_chunk.append(np.flatnonzero(near[:, dst].any(axis=1)))
        raw1.append(per_chunk)
    t1 = tuple(max(128, (max(len(raw1[c][ci]) for c in range(N_CORES))
                         + 127) // 128 * 128) // 128
               for ci in range(ICT))
    l1sels = []
    for c in range(N_CORES):
        parts = []
        for ci in range(ICT):
            src = raw1[c][ci]
            parts.append(np.concatenate(
                [src, np.full(t1[ci] * 128 - len(src), PAD_NODE,
                              src.dtype)]))
        l1sels.append(np.concatenate(parts))
    for c in range(N_CORES):
        l = np.concatenate([lists[c], halos[c]])
        lists[c] = np.concatenate(
            [l, np.full(nslot * 128 - len(l), PAD_NODE, l.dtype)])
    return order, p_sorted, lists, nslot, agpos, bp, l1sels, t1


def _prep_inputs(pos, pos_non_manifold, W1, a_src1, a_dst1, b1,
                 W2, a_src2, a_dst2, b2, fc_w, fc_b):
    bf16 = ml_dtypes.bfloat16
    pts = np.concatenate([np.asarray(pos, np.float32),
                          np.asarray(pos_non_manifold, np.float32)],
                         axis=2)[0].T  # [K, 3]
    order, p_sorted, lists, nslot, agpos, bp, l1sels, t1 = _plan(pts)
    sq_sorted = (p_sorted ** 2).sum(-1, dtype=np.float32).astype(np.float32)

    def bcast128(v):
        v = np.asarray(v, np.float32).reshape(-1)
        return np.ascontiguousarray(
            np.broadcast_to(v[None, :], (128, v.size)))

    def blockdiag(a):  # [H, C] -> [HC, H] fp32
        m = np.zeros((HC, H), dtype=np.float32)
        for h in range(H):
            m[h * C:(h + 1) * C, h] = np.asarray(a, np.float32)[h]
        return m

    def sel5_of(sel):
        psel = p_sorted[sel]
        return np.ascontiguousarray(np.concatenate(
            [psel.T, sq_sorted[sel][None, :],
             np.ones((1, len(sel)), np.float32)], axis=0)
            .astype(np.float32))

    W1f = np.asarray(W1, np.float32)
    W2f = np.asarray(W2, np.float32)
    w1p = np.concatenate([W1f, W1f @ blockdiag(a_src1)], axis=1)
    w2p = np.concatenate([W2f, W2f @ blockdiag(a_src2)], axis=1)

    shared = {
        "w1p": np.ascontiguousarray(w1p.astype(np.float32)),
        "w2p": np.ascontiguousarray(w2p.astype(bf16)),
        "adm1": blockdiag(a_dst1).astype(bf16),
        "adm2": blockdiag(a_dst2).astype(bf16),
        "bias1": bcast128(b1),
        "bias2": bcast128(b2),
        "fcw": bcast128(np.asarray(fc_w, np.float32).T),
        "fcb": bcast128(fc_b),
        "ident": np.eye(128, dtype=np.float32),
    }
    in_maps = []
    for c in range(N_CORES):
        sel = lists[c]
        pown = p_sorted[sel[:IC]]                 # own nodes, boundary-first
        own5 = np.concatenate(
            [-2.0 * pown.T, np.ones((1, IC), np.float32),
             (pown ** 2).sum(-1, dtype=np.float32)[None, :]], axis=0)
        # halo slots index boundary-layout ag rows via the host map
        agidx = np.ascontiguousarray(
            agpos[sel[OWN_SLOTS * 128:]].reshape(-1, 128).T
            .astype(np.int32))
        m = dict(shared)
        m["pts_sel5"] = sel5_of(sel)
        m["pts1"] = sel5_of(l1sels[c])
        m["pts_own5"] = np.ascontiguousarray(own5.astype(np.float32))
        m["pts_own3"] = np.ascontiguousarray(pown.T)
        m["agidx"] = agidx
        in_maps.append(m)
    return in_maps, order, nslot, lists, bp, t1


def kernel(pos, pos_non_manifold, W1, a_src1, a_dst1, b1,
           W2, a_src2, a_dst2, b2, fc_w, fc_b, _trace=False):
    in_maps, order, nslot, lists, bp, t1 = _prep_inputs(
        pos, pos_non_manifold, W1, a_src1, a_dst1, b1,
        W2, a_src2, a_dst2, b2, fc_w, fc_b)
    nc = _get_nc(nslot, bp, t1)
    res = run_bass_kernel_spmd(nc, in_maps, core_ids=list(range(N_CORES)),
                               trace=_trace)
    kernel.last_results = res
    x2s = np.concatenate([res.results[c]["out"] for c in range(N_CORES)],
                         axis=0)  # [KP, 2], rows in per-core list order
    perm = np.concatenate([lists[c][:IC] for c in range(N_CORES)])
    x2p = np.empty((KP, 2), np.float32)
    x2p[perm] = x2s
    x2 = np.empty((K, 2), np.float32)
    x2[order] = x2p[:K]
    logits = np.ascontiguousarray(x2[M:K]).reshape(1, 2, 3000)
    return logits.astype(np.float32)


# revision 69
# speedup vs baseline: 1.2721x; 1.2071x over previous
"""Trainium2 Bass kernel for a 2-layer GAT occupancy predictor (B=1).

Reference math:
  pts = concat(pos, pos_non_manifold) -> [K=6000, 3]
  mask[i,j] = ||pts_i - pts_j||^2 < 0.05^2          (dense radius graph)
  layer l:  h = x @ Wl                              [K, 4*64]
            e[i,j,h] = leaky02(ed[i,h] + es[j,h])   es/ed = <h, a_src/dst>
            alpha = softmax_j(e masked)
            x' = relu(alpha @ h + b)
  logits = (x2 @ fc_w + fc_b)[M:] reshaped to [1, 2, 3000]

Distribution (8 NeuronCores): nodes are Morton-sorted on the host so the
radius graph becomes block-local; core c owns destination rows
[768c, 768(c+1)) of the sorted, padded 6144-node graph.

Layer 1 is fully static and runs per destination chunk: for each of the 6
own 128-node chunks the host compacts the ~200 in-radius source nodes into
2-3 tiles (padded per chunk position across cores), and the score pipeline
runs merged over the chunk's tiles and all 4 heads via stride-0 broadcast
APs (one add, one 0.2x, one max, one mask-add, one exp per chunk).

Layer 2 runs per whole-core source slot: the ~870 unique sources are
compacted into 8 slots of 128 ordered [own 768 | halo | pad].  Each core's
own nodes are permuted boundary-first so the single bf16 AllGather between
layers only ships the rows other cores fetch; layer 2 computes own-slot h
from the local transposed features and fetches the halo slots' x rows with
one indirect DMA per slot, transposing on the PE.  The halo chain is
pushed late in the static schedule (tile_set_cur_wait) so the in-order
engine queues don't stall on the collective.  Layer-2 slot masks are
computed from static coordinates during the inter-layer window, filling
the AllGather bubble.

Engine mapping:
  PE   : h (with es riding along as 4 extra host-folded weight columns),
         d2 = |p_i - p_j|^2 as K=5 matmuls ([p; sq; 1] x [-2p; 1; sq]),
         the alpha @ h aggregation as bf16 hi+lo pairs into shared PSUM
         banks, denominators as N=1 ones-column matmuls.
  DVE  : scores, leaky, mask-adds (broadcast tensor_tensor), h hi/lo
         split, finalize division/relu, fc.
  ACT  : layer-2 head-0 leaky via Prelu, exp, PSUM->SBUF copies.
  Pool : partition-broadcast of ed, indirect halo gathers, AllGather.
Mask offsets (-30/0 bf16): per-pair tiles in layer 1, SBUF-resident
per-slot planes for layer 2.
Padded nodes sit at (-1,-1,-1): finite features, outside every real radius.
"""

import sys

sys.path.insert(0, "/opt/trn_rl_repo")

from contextlib import ExitStack

import ml_dtypes
import numpy as np

import concourse.bacc as bacc
import concourse.bass as bass
import concourse.mybir as mybir
import concourse.tile as tile
from concourse.bass_utils import run_bass_kernel_spmd

F32 = mybir.dt.float32
BF16 = mybir.dt.bfloat16
I32 = mybir.dt.int32
AF = mybir.ActivationFunctionType
OP = mybir.AluOpType
AX = mybir.AxisListType

N_CORES = 8
N = 3000
M = 3000
K = N + M          # real nodes
KP = 6144          # padded nodes
NT = KP // 128     # 48
IC = KP // N_CORES # 768 destinations per core
ICT = IC // 128    # 6 destination chunks per core
OWN_SLOTS = ICT    # first 6 layer-2 slots are the core's own nodes
H = 4              # heads
C = 64             # channels per head
HC = H * C         # 256
HCE = HC + H       # h columns + es columns
CP1 = C + 1        # head channels + ones column
R2 = float(np.float32(0.05) * np.float32(0.05))
PAD_COORD = -1.0
MASK_EPS = 1e-5    # host activity-test margin (superset of device mask)
MNEG = -30.0       # masked-score offset: exp(-30+L) ~ 1e-12
PAD_NODE = KP - 1  # all-padding node, used for unused slot entries


def build(nslot, bp, t1, n_cores=N_CORES, fake_ag=False):
    nc = bacc.Bacc("TRN2", target_bir_lowering=False, debug=False,
                   num_devices=n_cores)
    NS = nslot
    NH = NS - OWN_SLOTS          # halo slots (gathered in layer 2)
    BP = bp                      # boundary rows shipped in the AllGather
    NT1 = sum(t1)                # layer-1 chunk-tile count
    T1MAX = max(t1)
    assert NH >= 1 and BP % 128 == 0 and len(t1) == ICT

    # ---- kernel I/O (identical program on every core) ----
    # *sel5 rows: [p(3); sq; ones] for sources
    # pts_own5 rows: [-2p(3); ones; sq] for the own destination columns
    pts_sel5_d = nc.dram_tensor("pts_sel5", [5, NS * 128], F32,
                                kind="ExternalInput")
    pts1_d = nc.dram_tensor("pts1", [5, NT1 * 128], F32,
                            kind="ExternalInput")
    pts_own5_d = nc.dram_tensor("pts_own5", [5, IC], F32,
                                kind="ExternalInput")
    pts_own3_d = nc.dram_tensor("pts_own3", [3, IC], F32,
                                kind="ExternalInput")
    agidx_d = nc.dram_tensor("agidx", [128, NH], I32, kind="ExternalInput")
    # w1p/w2p: [W | W @ a_src_blockdiag] so es rides along with h
    w1p_d = nc.dram_tensor("w1p", [3, HCE], F32, kind="ExternalInput")
    w2p_d = nc.dram_tensor("w2p", [HC, HCE], BF16, kind="ExternalInput")
    adm1_d = nc.dram_tensor("adm1", [HC, H], BF16, kind="ExternalInput")
    adm2_d = nc.dram_tensor("adm2", [HC, H], BF16, kind="ExternalInput")
    bias1_d = nc.dram_tensor("bias1", [128, HC], F32, kind="ExternalInput")
    bias2_d = nc.dram_tensor("bias2", [128, HC], F32, kind="ExternalInput")
    fcw_d = nc.dram_tensor("fcw", [128, 2 * HC], F32, kind="ExternalInput")
    fcb_d = nc.dram_tensor("fcb", [128, 2], F32, kind="ExternalInput")
    ident_d = nc.dram_tensor("ident", [128, 128], F32, kind="ExternalInput")

    out_d = nc.dram_tensor("out", [IC, 2], F32, kind="ExternalOutput")

    with tile.TileContext(nc) as tc, ExitStack() as st:
        dram = st.enter_context(tc.tile_pool(name="dram", bufs=1,
                                             space="DRAM"))
        x_bounce = dram.tile([BP, HC], BF16)
        ag_out = dram.tile([n_cores * BP, HC], BF16,
                           addr_space=("Local" if fake_ag else "Shared"))

        const = st.enter_context(tc.tile_pool(name="const", bufs=1))
        pts_sel5_sb = const.tile([5, NS * 128], F32)
        pts1_sb = const.tile([5, NT1 * 128], F32)
        pts_own5_sb = const.tile([5, IC], F32)
        pts_own3_sb = const.tile([3, IC], F32)
        agidx_sb = const.tile([128, NH], I32)
        w1p_sb = const.tile([3, HCE], F32)
        w2p_sb = const.tile([128, 2, HCE], BF16)
        adm1_sb = const.tile([128, 2, H], BF16)
        adm2_sb = const.tile([128, 2, H], BF16)
        bias1_sb = const.tile([128, HC], F32)
        bias2_sb = const.tile([128, HC], F32)
        fcw_sb = const.tile([128, 2 * HC], F32)
        fcb_sb = const.tile([128, 2], F32)
        ident_sb = const.tile([128, 128], F32)
        ident_bf = const.tile([128, 128], BF16)
        w1b = const.tile([3, HCE], BF16)
        own3b = const.tile([3, IC], BF16)
        pts1b = const.tile([3, NT1 * 128], BF16)

        nc.sync.dma_start(out=pts_sel5_sb[:, :], in_=pts_sel5_d[:, :])
        nc.sync.dma_start(out=pts1_sb[:, :], in_=pts1_d[:, :])
        nc.sync.dma_start(out=pts_own5_sb[:, :], in_=pts_own5_d[:, :])
        nc.sync.dma_start(out=pts_own3_sb[:, :], in_=pts_own3_d[:, :])
        nc.sync.dma_start(out=agidx_sb[:, :], in_=agidx_d[:, :])
        nc.sync.dma_start(out=w1p_sb[:, :], in_=w1p_d[:, :])
        nc.sync.dma_start(out=w2p_sb[:, :, :],
                          in_=w2p_d.rearrange("(s p) c -> p s c", p=128))
        nc.sync.dma_start(out=adm1_sb[:, :, :],
                          in_=adm1_d.rearrange("(s p) h -> p s h", p=128))
        nc.sync.dma_start(out=adm2_sb[:, :, :],
                          in_=adm2_d.rearrange("(s p) h -> p s h", p=128))
        nc.sync.dma_start(out=bias1_sb[:, :], in_=bias1_d[:, :])
        nc.sync.dma_start(out=bias2_sb[:, :], in_=bias2_d[:, :])
        nc.sync.dma_start(out=fcw_sb[:, :], in_=fcw_d[:, :])
        nc.sync.dma_start(out=fcb_sb[:, :], in_=fcb_d[:, :])
        nc.sync.dma_start(out=ident_sb[:, :], in_=ident_d[:, :])
        nc.scalar.copy(ident_bf[:, :], ident_sb[:, :])
        nc.scalar.copy(w1b[:, :], w1p_sb[:, :])
        nc.scalar.copy(own3b[:, :], pts_own3_sb[:, :])
        nc.scalar.copy(pts1b[:, :], pts1_sb[0:3, :])

        big = st.enter_context(tc.tile_pool(name="big", bufs=1))
        hp1_hi = big.tile([128, NT1, H, CP1], BF16)
        hp1_lo = big.tile([128, NT1, H, CP1], BF16)
        es41 = big.tile([128, NT1, H], BF16)
        hp_hi = big.tile([128, NS, H, CP1], BF16)
        hp_lo = big.tile([128, NS, H, CP1], BF16)
        es4 = big.tile([128, NS, H], F32)
        ed_b = big.tile([128, H, IC], BF16)
        x_sb = big.tile([128, ICT, HC], F32)
        x_bf = big.tile([128, ICT, HC], BF16)
        xt_own = big.tile([128, 2, IC], BF16)
        edt_sb = big.tile([H, IC], BF16)
        edt_rows = big.tile([1, H, IC], BF16)
        logit_sb = big.tile([128, ICT, 2], F32)
        mn_sb = big.tile([128, NS, IC], BF16)   # layer-2 slot mask offsets

        nc.vector.memset(hp1_hi[:, :, :, C:CP1], 1.0)
        nc.vector.memset(hp1_lo[:, :, :, C:CP1], 0.0)
        nc.vector.memset(hp_hi[:, :, :, C:CP1], 1.0)
        nc.vector.memset(hp_lo[:, :, :, C:CP1], 0.0)

        # ================= layer 1 (per destination chunk) ==============
        # ---- own-column side: hT(own), edT, ED broadcasts ----
        with tc.tile_pool(name="prep1", bufs=2) as prep, \
             tc.tile_pool(name="prep_ps1", bufs=1, space="PSUM") as prep_ps:
            ht_own = prep.tile([128, 2, IC], BF16, tag="ht", bufs=1)
            for oc in range(2):
                ht_ps = prep_ps.tile([128, IC], F32, tag="ht_ps", bufs=1,
                                     name=f"ht_ps_1_{oc}")
                for lo, sz in ((0, 512), (512, 256)):
                    sl = slice(lo, lo + sz)
                    nc.tensor.matmul(
                        ht_ps[:, sl], w1b[:, oc * 128:(oc + 1) * 128],
                        own3b[:, sl], start=True, stop=True)
                nc.scalar.copy(ht_own[:, oc, :], ht_ps[:, :])

            edt_ps = prep_ps.tile([H, IC], F32, tag="edt", bufs=1)
            for s in range(2):
                for lo, sz in ((0, 512), (512, 256)):
                    sl = slice(lo, lo + sz)
                    nc.tensor.matmul(edt_ps[:, sl], adm1_sb[:, s, :],
                                     ht_own[:, s, sl],
                                     start=(s == 0), stop=(s == 1))
            nc.scalar.copy(edt_sb[:, :], edt_ps[:, :])
            for h in range(H):
                nc.sync.dma_start(out=edt_rows[0:1, h, :],
                                  in_=edt_sb[h:h + 1, :])
            # stagger: first chunks' columns land before the full rows so
            # the chunk loop can start scoring sooner
            for h in range(H):
                nc.gpsimd.partition_broadcast(ed_b[:, h, 0:256],
                                              edt_rows[0:1, h, 0:256])
            for h in range(H):
                nc.gpsimd.partition_broadcast(ed_b[:, h, 256:IC],
                                              edt_rows[0:1, h, 256:IC])

        # ---- chunk loop: h+es, per-pair masks, merged scores, agg ----
        with tc.tile_pool(name="agg_ps1", bufs=1, space="PSUM") as agg_ps:
            aggp = [agg_ps.tile([128, 2, H, C], F32, tag=f"agg{p}",
                                name=f"agg_1_{p}")
                    for p in range(ICT // 2)]
            den_ps = agg_ps.tile([128, ICT, H], F32, tag="den",
                                 name="den_1")
            with tc.tile_pool(name="jl1", bufs=3) as jl, \
                 tc.tile_pool(name="h_ps1", bufs=2, space="PSUM") as h_psp:
                # pass 1: per-tile h/es/hi-lo/masks (independent of ed_b,
                # fills the DVE queue while the prep broadcasts run)
                mnCs = []
                tbase = 0
                for ci in range(ICT):
                    Tc = t1[ci]
                    csl = slice(ci * 128, (ci + 1) * 128)
                    mnC = jl.tile([128, T1MAX, 128], BF16, tag="mn",
                                  bufs=ICT, name=f"mn1_{ci}")
                    mnCs.append(mnC)
                    for t in range(Tc):
                        s = tbase + t
                        ssl = slice(s * 128, (s + 1) * 128)
                        h_ps = h_psp.tile([128, HCE], F32, tag="h",
                                          name=f"h1_{s}")
                        nc.tensor.matmul(h_ps[:, :], pts1b[:, ssl],
                                         w1b[:, :],
                                         start=True, stop=True)
                        nc.vector.tensor_scalar_add(
                            es41[:, s, :], h_ps[:, HC:HCE], 0.0)
                        nc.scalar.copy(
                            hp1_hi[:, s, :, 0:C],
                            h_ps[:, 0:HC].rearrange("p (h c) -> p h c",
                                                    h=H))
                        nc.vector.tensor_tensor(
                            hp1_lo[:, s, :, 0:C],
                            h_ps[:, 0:HC].rearrange("p (h c) -> p h c",
                                                    h=H),
                            hp1_hi[:, s, :, 0:C], OP.subtract)
                        g_ps = h_psp.tile([128, 128], F32, tag="g",
                                          bufs=2, name=f"g1_{s}")
                        nc.tensor.matmul(g_ps[:, :], pts1_sb[:, ssl],
                                         pts_own5_sb[:, csl],
                                         start=True, stop=True)
                        nc.vector.tensor_scalar(
                            mnC[:, t, :], g_ps[:, :], R2, MNEG,
                            OP.is_ge, OP.mult)
                    tbase += Tc

                # pass 2: merged scores + aggregation per chunk
                tbase = 0
                for ci in range(ICT):
                    Tc = t1[ci]
                    csl = slice(ci * 128, (ci + 1) * 128)
                    mnC = mnCs[ci]
                    L = jl.tile([128, Tc, H, 128], BF16, tag="L4",
                                name=f"L1_{ci}")
                    e1 = ed_b[:, :, csl].rearrange(
                        "p (o h) d -> p o h d", o=1)
                    e2 = es41[:, tbase:tbase + Tc, :].rearrange(
                        "p t (h o) -> p t h o", o=1)
                    b1, b2 = bass.broadcast_tensor_aps(e1, e2)
                    nc.vector.tensor_tensor(L[:, :, :, :], b1, b2, OP.add)
                    T4 = jl.tile([128, Tc, H, 128], BF16, tag="T4",
                                 name=f"T4_1_{ci}")
                    nc.vector.tensor_scalar(T4[:, :, :, :], L[:, :, :, :],
                                            0.2, None, OP.mult)
                    nc.vector.tensor_tensor(L[:, :, :, :], L[:, :, :, :],
                                            T4[:, :, :, :], OP.max)
                    mb = mnC[:, 0:Tc, :].rearrange(
                        "p t (o d) -> p t o d", o=1)
                    bl, bm = bass.broadcast_tensor_aps(L[:, :, :, :], mb)
                    nc.vector.tensor_tensor(L[:, :, :, :], bl, bm, OP.add)
                    A = jl.tile([128, Tc, H, 128], BF16, tag="A4",
                                name=f"A1_{ci}")
                    if ci == ICT - 1:
                        nc.scalar.activation(A[:, 0:1, :, :],
                                             L[:, 0:1, :, :], AF.Exp)
                        nc.scalar.activation(A[:, 1:Tc, :, :],
                                             L[:, 1:Tc, :, :], AF.Exp)
                    else:
                        nc.scalar.activation(A[:, :, :, :], L[:, :, :, :],
                                             AF.Exp)

                    bank = aggp[ci // 2]
                    half = ci % 2
                    for t in range(Tc):
                        s = tbase + t
                        for h in range(H):
                            first = (half == 0 and t == 0 and h == 0)
                            last = (half == 1 and t == Tc - 1
                                    and h == H - 1)
                            nc.tensor.matmul(
                                bank[:, half, h, :], A[:, t, h, :],
                                hp1_hi[:, s, h, 0:C],
                                start=first, stop=False)
                            nc.tensor.matmul(
                                bank[:, half, h, :], A[:, t, h, :],
                                hp1_lo[:, s, h, 0:C],
                                start=False, stop=last)
                            nc.tensor.matmul(
                                den_ps[:, ci, h:h + 1], A[:, t, h, :],
                                hp1_hi[:, s, h, C:CP1],
                                start=(ci == 0 and t == 0 and h == 0),
                                stop=(ci == ICT - 1 and t == Tc - 1
                                      and h == H - 1))
                    tbase += Tc

            # ---- finalize x1, AllGather, transposes, layer-2 masks ----
            with tc.tile_pool(name="fin1", bufs=2) as fin, \
                 tc.tile_pool(name="fin_ps1", bufs=2,
                              space="PSUM") as fin_ps:
                qb = BP // 128
                for ic in range(ICT):
                    rec = fin.tile([128, H], F32, tag="rec",
                                   name=f"rec_1_{ic}")
                    nc.vector.reciprocal(rec[:, :], den_ps[:, ic, :])
                    for h in range(H):
                        nc.vector.scalar_tensor_tensor(
                            x_sb[:, ic, h * C:(h + 1) * C],
                            aggp[ic // 2][:, ic % 2, h, :],
                            rec[:, h:h + 1],
                            bias1_sb[:, h * C:(h + 1) * C],
                            OP.mult, OP.add)
                    if ic == qb - 1:
                        # boundary chunks done: fire the AllGather now,
                        # before the rest of the finalize
                        nc.vector.tensor_scalar(
                            x_sb[:, 0:qb, :], x_sb[:, 0:qb, :],
                            0.0, None, OP.max)
                        nc.scalar.copy(x_bf[:, 0:qb, :],
                                       x_sb[:, 0:qb, :])
                        nc.sync.dma_start(
                            out=x_bounce.rearrange("(q p) c -> p q c",
                                                   p=128),
                            in_=x_bf[:, 0:qb, :])
                        if fake_ag:
                            for r in range(n_cores):
                                nc.sync.dma_start(
                                    out=ag_out[r * BP:(r + 1) * BP, :],
                                    in_=x_bounce[:, :])
                        else:
                            nc.gpsimd.collective_compute(
                                "AllGather", OP.bypass,
                                replica_groups=[list(range(n_cores))],
                                ins=[x_bounce.opt()],
                                outs=[ag_out.opt()])
                nc.vector.tensor_scalar(x_sb[:, qb:, :], x_sb[:, qb:, :],
                                        0.0, None, OP.max)
                # transposed own x, needed for layer-2 prep
                for ic in range(ICT):
                    for oc in range(2):
                        t_ps = fin_ps.tile([128, 128], F32, tag="t_ps",
                                           name=f"t_ps_{ic}_{oc}")
                        nc.tensor.transpose(
                            t_ps[:, :],
                            x_sb[:, ic, oc * 128:(oc + 1) * 128],
                            ident_sb[:, :])
                        nc.scalar.copy(
                            xt_own[:, oc, ic * 128:(ic + 1) * 128],
                            t_ps[:, :])
                # layer-2 slot masks from static coords: fills the
                # AllGather bubble
                with tc.tile_pool(name="mn_ps2", bufs=1,
                                  space="PSUM") as mn_psp:
                    for s in range(NS):
                        for lo, sz in ((0, 512), (512, 256)):
                            sl = slice(lo, lo + sz)
                            g2 = mn_psp.tile([128, sz], F32,
                                             tag=f"g{lo}", bufs=1,
                                             name=f"g2_{s}_{lo}")
                            nc.tensor.matmul(
                                g2[:, :],
                                pts_sel5_sb[:, s * 128:(s + 1) * 128],
                                pts_own5_sb[:, sl],
                                start=True, stop=True)
                            nc.vector.tensor_scalar(
                                mn_sb[:, s, sl], g2[:, :], R2, MNEG,
                                OP.is_ge, OP.mult)

        # ================= layer 2 (per source slot) ====================
        with tc.tile_pool(name="prep2", bufs=2) as prep, \
             tc.tile_pool(name="prep_ps2", bufs=1, space="PSUM") as prep_ps:
            ht_own = prep.tile([128, 2, IC], BF16, tag="ht", bufs=1)
            for oc in range(2):
                ht_ps = prep_ps.tile([128, IC], F32, tag="ht_ps", bufs=1,
                                     name=f"ht_ps_2_{oc}")
                for s in range(2):
                    for lo, sz in ((0, 512), (512, 256)):
                        sl = slice(lo, lo + sz)
                        nc.tensor.matmul(
                            ht_ps[:, sl],
                            w2p_sb[:, s, oc * 128:(oc + 1) * 128],
                            xt_own[:, s, sl],
                            start=(s == 0), stop=(s == 1))
                nc.scalar.copy(ht_own[:, oc, :], ht_ps[:, :])

            edt_ps = prep_ps.tile([H, IC], F32, tag="edt", bufs=1)
            for s in range(2):
                for lo, sz in ((0, 512), (512, 256)):
                    sl = slice(lo, lo + sz)
                    nc.tensor.matmul(edt_ps[:, sl], adm2_sb[:, s, :],
                                     ht_own[:, s, sl],
                                     start=(s == 0), stop=(s == 1))
            nc.scalar.copy(edt_sb[:, :], edt_ps[:, :])
            for h in range(H):
                nc.sync.dma_start(out=edt_rows[0:1, h, :],
                                  in_=edt_sb[h:h + 1, :])
            for h in range(H):
                nc.gpsimd.partition_broadcast(ed_b[:, h, :],
                                              edt_rows[0:1, h, :])

        with tc.tile_pool(name="agg_ps2", bufs=1, space="PSUM") as agg_ps:
            aggp = [agg_ps.tile([128, 2, H, C], F32, tag=f"agg{p}",
                                name=f"agg_2_{p}")
                    for p in range(ICT // 2)]
            den_ps = agg_ps.tile([128, ICT, H], F32, tag="den",
                                 name="den_2")
            with tc.tile_pool(name="jl2", bufs=3) as jl, \
                 tc.tile_pool(name="h_ps2", bufs=2, space="PSUM") as h_psp:
                for s in range(NS):
                    # halo slots wait on the AllGather: tell the static
                    # scheduler to order their chain after the own slots
                    # so in-order engine queues don't stall behind it
                    if s >= OWN_SLOTS:
                        tc.tile_set_cur_wait(0.5)
                    # --- h + es for this slot's 128 sources ---
                    h_ps = h_psp.tile([128, HCE], F32, tag="h",
                                      name=f"h_ps_2_{s}")
                    if s < OWN_SLOTS:
                        for half in range(2):
                            nc.tensor.matmul(
                                h_ps[:, :],
                                xt_own[:, half, s * 128:(s + 1) * 128],
                                w2p_sb[:, half, :],
                                start=(half == 0), stop=(half == 1))
                    else:
                        xg = jl.tile([128, HC], BF16, tag="xg",
                                     name=f"xg_2_{s}")
                        nc.gpsimd.indirect_dma_start(
                            out=xg[:, :], out_offset=None,
                            in_=ag_out,
                            in_offset=bass.IndirectOffsetOnAxis(
                                ap=agidx_sb[:, s - OWN_SLOTS:
                                            s - OWN_SLOTS + 1],
                                axis=0))
                        xtg = jl.tile([128, 2, 128], BF16, tag="xtg",
                                      name=f"xtg_2_{s}")
                        for half in range(2):
                            t_ps = h_psp.tile(
                                [128, 128], BF16, tag=f"tr{half}",
                                bufs=1, name=f"tr_2_{s}_{half}")
                            nc.tensor.transpose(
                                t_ps[:, :],
                                xg[:, half * 128:(half + 1) * 128],
                                ident_bf[:, :])
                            nc.scalar.copy(xtg[:, half, :], t_ps[:, :])
                        for half in range(2):
                            nc.tensor.matmul(
                                h_ps[:, :], xtg[:, half, :],
                                w2p_sb[:, half, :],
                                start=(half == 0), stop=(half == 1))
                    nc.vector.tensor_scalar_add(es4[:, s, :],
                                                h_ps[:, HC:HCE], 0.0)
                    # h -> bf16 hi + lo with ones/zeros column
                    nc.scalar.copy(
                        hp_hi[:, s, :, 0:C],
                        h_ps[:, 0:HC].rearrange("p (h c) -> p h c", h=H))
                    nc.vector.tensor_tensor(
                        hp_lo[:, s, :, 0:C],
                        h_ps[:, 0:HC].rearrange("p (h c) -> p h c", h=H),
                        hp_hi[:, s, :, 0:C], OP.subtract)

                    # --- scores: L = leaky(ed+es) + mn ; A = exp(L) ---
                    L4 = jl.tile([128, H, IC], BF16, tag="L4",
                                 name=f"L4_2_{s}")
                    nc.scalar.activation(L4[:, 0, :], ed_b[:, 0, :],
                                         AF.Prelu,
                                         bias=es4[:, s, 0:1],
                                         scale=1.0, alpha=0.2)
                    T4 = jl.tile([128, 3, IC], BF16, tag="T4",
                                 name=f"T4_2_{s}")
                    for h in range(1, H):
                        nc.vector.tensor_scalar(
                            L4[:, h, :], ed_b[:, h, :],
                            es4[:, s, h:h + 1], None, OP.add)
                    nc.vector.tensor_scalar(
                        T4[:, :, :], L4[:, 1:4, :], 0.2, None, OP.mult)
                    nc.vector.tensor_tensor(L4[:, 1:4, :], L4[:, 1:4, :],
                                            T4[:, :, :], OP.max)
                    l4b, mnb = bass.broadcast_tensor_aps(
                        L4[:, :, :],
                        mn_sb[:, s:s + 1, :])
                    nc.vector.tensor_tensor(L4[:, :, :], l4b, mnb,
                                            OP.add)
                    A4 = jl.tile([128, H, IC], BF16, tag="A4",
                                 name=f"A4_2_{s}")
                    if s == NS - 1:
                        nc.scalar.activation(A4[:, 0:2, :], L4[:, 0:2, :],
                                             AF.Exp)
                        nc.scalar.activation(A4[:, 2:4, :], L4[:, 2:4, :],
                                             AF.Exp)
                    else:
                        nc.scalar.activation(A4[:, :, :], L4[:, :, :],
                                             AF.Exp)

                    # --- aggregation ---
                    for h in range(H):
                        for ic in range(ICT):
                            out_ap = aggp[ic // 2][:, ic % 2, h, :]
                            first = (s == 0 and h == 0 and ic % 2 == 0)
                            last = (s == NS - 1 and h == H - 1
                                    and ic % 2 == 1)
                            nc.tensor.matmul(
                                out_ap,
                                A4[:, h, ic * 128:(ic + 1) * 128],
                                hp_hi[:, s, h, 0:C],
                                start=first, stop=False)
                            nc.tensor.matmul(
                                out_ap,
                                A4[:, h, ic * 128:(ic + 1) * 128],
                                hp_lo[:, s, h, 0:C],
                                start=False, stop=last)
                            nc.tensor.matmul(
                                den_ps[:, ic, h:h + 1],
                                A4[:, h, ic * 128:(ic + 1) * 128],
                                hp_hi[:, s, h, C:CP1],
                                start=(s == 0 and h == 0 and ic == 0),
                                stop=(s == NS - 1 and h == H - 1
                                      and ic == ICT - 1))
                tc.cur_wait_ts = None

            # ---- finalize x2 = relu(num/den + b); fc ----
            with tc.tile_pool(name="fin2", bufs=2) as fin:
                for ic in range(ICT):
                    rec = fin.tile([128, H], F32, tag="rec",
                                   name=f"rec_2_{ic}")
                    nc.vector.reciprocal(rec[:, :], den_ps[:, ic, :])
                    for h in range(H):
                        nc.vector.scalar_tensor_tensor(
                            x_sb[:, ic, h * C:(h + 1) * C],
                            aggp[ic // 2][:, ic % 2, h, :],
                            rec[:, h:h + 1],
                            bias2_sb[:, h * C:(h + 1) * C],
                            OP.mult, OP.add)
                nc.vector.tensor_scalar(x_sb[:, :, :], x_sb[:, :, :],
                                        0.0, None, OP.max)
                for ic in range(ICT):
                    for o in range(2):
                        prod = fin.tile([128, HC], F32, tag="prod",
                                        name=f"prod_{ic}_{o}")
                        nc.vector.tensor_tensor(
                            prod[:, :], x_sb[:, ic, :],
                            fcw_sb[:, o * HC:(o + 1) * HC], OP.mult)
                        red = fin.tile([128, 1], F32, tag="red",
                                       name=f"red_{ic}_{o}")
                        nc.vector.tensor_reduce(
                            red[:, :], prod[:, :], AX.X, OP.add)
                        nc.vector.tensor_scalar_add(
                            logit_sb[:, ic, o:o + 1], red[:, :],
                            fcb_sb[:, o:o + 1])
                nc.sync.dma_start(
                    out=out_d.rearrange("(q p) o -> p q o", p=128),
                    in_=logit_sb[:, :, :])

    nc.compile()
    return nc


_BUILD_CACHE = {}


def _get_nc(nslot, bp, t1):
    key = (nslot, bp, t1)
    if key not in _BUILD_CACHE:
        _BUILD_CACHE[key] = build(nslot, bp, t1)
    return _BUILD_CACHE[key]


def _morton(p, bits=10):
    q = np.clip((p * (1 << bits)).astype(np.int64), 0, (1 << bits) - 1)
    code = np.zeros(len(p), np.int64)
    for b in range(bits):
        for dim in range(3):
            code |= ((q[:, dim] >> b) & 1) << (3 * b + dim)
    return code


def _plan(pts):
    """Sort nodes spatially, build compacted source lists for both layers."""
    order = np.argsort(_morton(pts), kind="stable")
    p_sorted = np.full((KP, 3), PAD_COORD, np.float32)
    p_sorted[:K] = pts[order]

    sq = (p_sorted ** 2).sum(-1, dtype=np.float32)
    G = p_sorted @ p_sorted.T
    d2 = sq[None, :] + sq[:, None] - 2.0 * G
    near = d2 < (R2 + MASK_EPS)          # [j, i], conservative superset

    halos = []
    for c in range(N_CORES):
        act = np.flatnonzero(near[:, c * IC:(c + 1) * IC].any(axis=1))
        halos.append(act[(act < c * IC) | (act >= (c + 1) * IC)])
    nslot = max(OWN_SLOTS + 1,
                max(OWN_SLOTS + (len(h) + 127) // 128 for h in halos))
    # permute each core's own nodes boundary-first so the AllGather only
    # ships the rows other cores actually fetch
    boundary = np.zeros(KP, bool)
    for h in halos:
        boundary[h] = True
    lists, agpos = [], np.zeros(KP, np.int64)
    bnd_counts = []
    for c in range(N_CORES):
        own = np.arange(c * IC, (c + 1) * IC)
        isb = boundary[own]
        perm = np.concatenate([own[isb], own[~isb]])
        bnd_counts.append(int(isb.sum()))
        lists.append(perm)
    bp = max(128, 128 * ((max(bnd_counts) + 127) // 128))
    for c in range(N_CORES):
        agpos[lists[c][:bnd_counts[c]]] = c * bp + np.arange(bnd_counts[c])
    # layer-1 per-chunk compacted source tiles (chunks in permuted order)
    raw1 = []
    for c in range(N_CORES):
        per_chunk = []
        for ci in range(ICT):
            dst = lists[c][ci * 128:(ci + 1) * 128]
            per_chunk.append(np.flatnonzero(near[:, dst].any(axis=1)))
        raw1.append(per_chunk)
    t1 = tuple(max(128, (max(len(raw1[c][ci]) for c in range(N_CORES))
                         + 127) // 128 * 128) // 128
               for ci in range(ICT))
    l1sels = []
    for c in range(N_CORES):
        parts = []
        for ci in range(ICT):
            src = raw1[c][ci]
            parts.append(np.concatenate(
                [src, np.full(t1[ci] * 128 - len(src), PAD_NODE,
                              src.dtype)]))
        l1sels.append(np.concatenate(parts))
    for c in range(N_CORES):
        l = np.concatenate([lists[c], halos[c]])
        lists[c] = np.concatenate(
            [l, np.full(nslot * 128 - len(l), PAD_NODE, l.dtype)])
    return order, p_sorted, lists, nslot, agpos, bp, l1sels, t1


def _prep_inputs(pos, pos_non_manifold, W1, a_src1, a_dst1, b1,
                 W2, a_src2, a_dst2, b2, fc_w, fc_b):
    bf16 = ml_dtypes.bfloat16
    pts = np.concatenate([np.asarray(pos, np.float32),
                          np.asarray(pos_non_manifold, np.float32)],
                         axis=2)[0].T  # [K, 3]
    order, p_sorted, lists, nslot, agpos, bp, l1sels, t1 = _plan(pts)
    sq_sorted = (p_sorted ** 2).sum(-1, dtype=np.float32).astype(np.float32)

    def bcast128(v):
        v = np.asarray(v, np.float32).reshape(-1)
        return np.ascontiguousarray(
            np.broadcast_to(v[None, :], (128, v.size)))

    def blockdiag(a):  # [H, C] -> [HC, H] fp32
        m = np.zeros((HC, H), dtype=np.float32)
        for h in range(H):
            m[h * C:(h + 1) * C, h] = np.asarray(a, np.float32)[h]
        return m

    def sel5_of(sel):
        psel = p_sorted[sel]
        return np.ascontiguousarray(np.concatenate(
            [psel.T, sq_sorted[sel][None, :],
             np.ones((1, len(sel)), np.float32)], axis=0)
            .astype(np.float32))

    W1f = np.asarray(W1, np.float32)
    W2f = np.asarray(W2, np.float32)
    w1p = np.concatenate([W1f, W1f @ blockdiag(a_src1)], axis=1)
    w2p = np.concatenate([W2f, W2f @ blockdiag(a_src2)], axis=1)

    shared = {
        "w1p": np.ascontiguousarray(w1p.astype(np.float32)),
        "w2p": np.ascontiguousarray(w2p.astype(bf16)),
        "adm1": blockdiag(a_dst1).astype(bf16),
        "adm2": blockdiag(a_dst2).astype(bf16),
        "bias1": bcast128(b1),
        "bias2": bcast128(b2),
        "fcw": bcast128(np.asarray(fc_w, np.float32).T),
        "fcb": bcast128(fc_b),
        "ident": np.eye(128, dtype=np.float32),
    }
    in_maps = []
    for c in range(N_CORES):
        sel = lists[c]
        pown = p_sorted[sel[:IC]]                 # own nodes, boundary-first
        own5 = np.concatenate(
            [-2.0 * pown.T, np.ones((1, IC), np.float32),
             (pown ** 2).sum(-1, dtype=np.float32)[None, :]], axis=0)
        # halo slots index boundary-layout ag rows via the host map
        agidx = np.ascontiguousarray(
            agpos[sel[OWN_SLOTS * 128:]].reshape(-1, 128).T
            .astype(np.int32))
        m = dict(shared)
        m["pts_sel5"] = sel5_of(sel)
        m["pts1"] = sel5_of(l1sels[c])
        m["pts_own5"] = np.ascontiguousarray(own5.astype(np.float32))
        m["pts_own3"] = np.ascontiguousarray(pown.T)
        m["agidx"] = agidx
        in_maps.append(m)
    return in_maps, order, nslot, lists, bp, t1


def kernel(pos, pos_non_manifold, W1, a_src1, a_dst1, b1,
           W2, a_src2, a_dst2, b2, fc_w, fc_b, _trace=False):
    in_maps, order, nslot, lists, bp, t1 = _prep_inputs(
        pos, pos_non_manifold, W1, a_src1, a_dst1, b1,
        W2, a_src2, a_dst2, b2, fc_w, fc_b)
    nc = _get_nc(nslot, bp, t1)
    res = run_bass_kernel_spmd(nc, in_maps, core_ids=list(range(N_CORES)),
                               trace=_trace)
    kernel.last_results = res
    x2s = np.concatenate([res.results[c]["out"] for c in range(N_CORES)],
                         axis=0)  # [KP, 2], rows in per-core list order
    perm = np.concatenate([lists[c][:IC] for c in range(N_CORES)])
    x2p = np.empty((KP, 2), np.float32)
    x2p[perm] = x2s
    x2 = np.empty((K, 2), np.float32)
    x2[order] = x2p[:K]
    logits = np.ascontiguousarray(x2[M:K]).reshape(1, 2, 3000)
    return logits.astype(np.float32)


# revision 70
# speedup vs baseline: 1.2833x; 1.0088x over previous
"""Trainium2 Bass kernel for a 2-layer GAT occupancy predictor (B=1).

Reference math:
  pts = concat(pos, pos_non_manifold) -> [K=6000, 3]
  mask[i,j] = ||pts_i - pts_j||^2 < 0.05^2          (dense radius graph)
  layer l:  h = x @ Wl                              [K, 4*64]
            e[i,j,h] = leaky02(ed[i,h] + es[j,h])   es/ed = <h, a_src/dst>
            alpha = softmax_j(e masked)
            x' = relu(alpha @ h + b)
  logits = (x2 @ fc_w + fc_b)[M:] reshaped to [1, 2, 3000]

Distribution (8 NeuronCores): nodes are Morton-sorted on the host so the
radius graph becomes block-local; core c owns destination rows
[768c, 768(c+1)) of the sorted, padded 6144-node graph.

Layer 1 is fully static and runs per destination chunk: for each of the 6
own 128-node chunks the host compacts the ~200 in-radius source nodes into
2-3 tiles (padded per chunk position across cores), and the score pipeline
runs merged over the chunk's tiles and all 4 heads via stride-0 broadcast
APs (one add, one 0.2x, one max, one mask-add, one exp per chunk).

Layer 2 runs per whole-core source slot: the ~870 unique sources are
compacted into 8 slots of 128 ordered [own 768 | halo | pad].  Each core's
own nodes are permuted boundary-first so the single bf16 AllGather between
layers only ships the rows other cores fetch; layer 2 computes own-slot h
from the local transposed features and fetches the halo slots' x rows with
one indirect DMA per slot, transposing on the PE.  The halo chain is
pushed late in the static schedule (tile_set_cur_wait) so the in-order
engine queues don't stall on the collective.  Layer-2 slot masks are
computed from static coordinates during the inter-layer window, filling
the AllGather bubble.

Engine mapping:
  PE   : h (with es riding along as 4 extra host-folded weight columns),
         d2 = |p_i - p_j|^2 as K=5 matmuls ([p; sq; 1] x [-2p; 1; sq]),
         the alpha @ h aggregation as bf16 hi+lo pairs into shared PSUM
         banks, denominators as N=1 ones-column matmuls.
  DVE  : scores, leaky, mask-adds (broadcast tensor_tensor), h hi/lo
         split, finalize division/relu, fc.
  ACT  : layer-2 head-0 leaky via Prelu, exp, PSUM->SBUF copies.
  Pool : partition-broadcast of ed, indirect halo gathers, AllGather.
Mask offsets (-30/0 bf16): per-pair tiles in layer 1, SBUF-resident
per-slot planes for layer 2.
Padded nodes sit at (-1,-1,-1): finite features, outside every real radius.
"""

import sys

sys.path.insert(0, "/opt/trn_rl_repo")

from contextlib import ExitStack

import ml_dtypes
import numpy as np

import concourse.bacc as bacc
import concourse.bass as bass
import concourse.mybir as mybir
import concourse.tile as tile
from concourse.bass_utils import run_bass_kernel_spmd

F32 = mybir.dt.float32
BF16 = mybir.dt.bfloat16
I32 = mybir.dt.int32
AF = mybir.ActivationFunctionType
OP = mybir.AluOpType
AX = mybir.AxisListType

N_CORES = 8
N = 3000
M = 3000
K = N + M          # real nodes
KP = 6144          # padded nodes
NT = KP // 128     # 48
IC = KP // N_CORES # 768 destinations per core
ICT = IC // 128    # 6 destination chunks per core
OWN_SLOTS = ICT    # first 6 layer-2 slots are the core's own nodes
H = 4              # heads
C = 64             # channels per head
HC = H * C         # 256
HCE = HC + H       # h columns + es columns
CP1 = C + 1        # head channels + ones column
R2 = float(np.float32(0.05) * np.float32(0.05))
PAD_COORD = -1.0
MASK_EPS = 1e-5    # host activity-test margin (superset of device mask)
MNEG = -30.0       # masked-score offset: exp(-30+L) ~ 1e-12
PAD_NODE = KP - 1  # all-padding node, used for unused slot entries


def build(nslot, bp, t1, n_cores=N_CORES, fake_ag=False):
    nc = bacc.Bacc("TRN2", target_bir_lowering=False, debug=False,
                   num_devices=n_cores)
    NS = nslot
    NH = NS - OWN_SLOTS          # halo slots (gathered in layer 2)
    BP = bp                      # boundary rows shipped in the AllGather
    NT1 = sum(t1)                # layer-1 chunk-tile count
    T1MAX = max(t1)
    assert NH >= 1 and BP % 128 == 0 and len(t1) == ICT

    # ---- kernel I/O (identical program on every core) ----
    # *sel5 rows: [p(3); sq; ones] for sources
    # pts_own5 rows: [-2p(3); ones; sq] for the own destination columns
    pts_sel5_d = nc.dram_tensor("pts_sel5", [5, NS * 128], F32,
                                kind="ExternalInput")
    pts1_d = nc.dram_tensor("pts1", [5, NT1 * 128], F32,
                            kind="ExternalInput")
    pts_own5_d = nc.dram_tensor("pts_own5", [5, IC], F32,
                                kind="ExternalInput")
    pts_own3_d = nc.dram_tensor("pts_own3", [3, IC], F32,
                                kind="ExternalInput")
    agidx_d = nc.dram_tensor("agidx", [128, NH], I32, kind="ExternalInput")
    # w1p/w2p: [W | W @ a_src_blockdiag] so es rides along with h
    w1p_d = nc.dram_tensor("w1p", [3, HCE], F32, kind="ExternalInput")
    w2p_d = nc.dram_tensor("w2p", [HC, HCE], BF16, kind="ExternalInput")
    adm1_d = nc.dram_tensor("adm1", [HC, H], BF16, kind="ExternalInput")
    adm2_d = nc.dram_tensor("adm2", [HC, H], BF16, kind="ExternalInput")
    bias1_d = nc.dram_tensor("bias1", [128, HC], F32, kind="ExternalInput")
    bias2_d = nc.dram_tensor("bias2", [128, HC], F32, kind="ExternalInput")
    fcw_d = nc.dram_tensor("fcw", [128, 2 * HC], F32, kind="ExternalInput")
    fcb_d = nc.dram_tensor("fcb", [128, 2], F32, kind="ExternalInput")
    ident_d = nc.dram_tensor("ident", [128, 128], F32, kind="ExternalInput")

    out_d = nc.dram_tensor("out", [IC, 2], F32, kind="ExternalOutput")

    with tile.TileContext(nc) as tc, ExitStack() as st:
        dram = st.enter_context(tc.tile_pool(name="dram", bufs=1,
                                             space="DRAM"))
        x_bounce = dram.tile([BP, HC], BF16)
        ag_out = dram.tile([n_cores * BP, HC], BF16,
                           addr_space=("Local" if fake_ag else "Shared"))

        const = st.enter_context(tc.tile_pool(name="const", bufs=1))
        pts_sel5_sb = const.tile([5, NS * 128], F32)
        pts1_sb = const.tile([5, NT1 * 128], F32)
        pts_own5_sb = const.tile([5, IC], F32)
        pts_own3_sb = const.tile([3, IC], F32)
        agidx_sb = const.tile([128, NH], I32)
        w1p_sb = const.tile([3, HCE], F32)
        w2p_sb = const.tile([128, 2, HCE], BF16)
        adm1_sb = const.tile([128, 2, H], BF16)
        adm2_sb = const.tile([128, 2, H], BF16)
        bias1_sb = const.tile([128, HC], F32)
        bias2_sb = const.tile([128, HC], F32)
        fcw_sb = const.tile([128, 2 * HC], F32)
        fcb_sb = const.tile([128, 2], F32)
        ident_sb = const.tile([128, 128], F32)
        ident_bf = const.tile([128, 128], BF16)
        w1b = const.tile([3, HCE], BF16)
        own3b = const.tile([3, IC], BF16)
        pts1b = const.tile([3, NT1 * 128], BF16)

        nc.sync.dma_start(out=pts_sel5_sb[:, :], in_=pts_sel5_d[:, :])
        nc.sync.dma_start(out=pts1_sb[:, :], in_=pts1_d[:, :])
        nc.sync.dma_start(out=pts_own5_sb[:, :], in_=pts_own5_d[:, :])
        nc.sync.dma_start(out=pts_own3_sb[:, :], in_=pts_own3_d[:, :])
        nc.sync.dma_start(out=agidx_sb[:, :], in_=agidx_d[:, :])
        nc.sync.dma_start(out=w1p_sb[:, :], in_=w1p_d[:, :])
        nc.sync.dma_start(out=w2p_sb[:, :, :],
                          in_=w2p_d.rearrange("(s p) c -> p s c", p=128))
        nc.sync.dma_start(out=adm1_sb[:, :, :],
                          in_=adm1_d.rearrange("(s p) h -> p s h", p=128))
        nc.sync.dma_start(out=adm2_sb[:, :, :],
                          in_=adm2_d.rearrange("(s p) h -> p s h", p=128))
        nc.sync.dma_start(out=bias1_sb[:, :], in_=bias1_d[:, :])
        nc.sync.dma_start(out=bias2_sb[:, :], in_=bias2_d[:, :])
        nc.sync.dma_start(out=fcw_sb[:, :], in_=fcw_d[:, :])
        nc.sync.dma_start(out=fcb_sb[:, :], in_=fcb_d[:, :])
        nc.sync.dma_start(out=ident_sb[:, :], in_=ident_d[:, :])
        nc.scalar.copy(ident_bf[:, :], ident_sb[:, :])
        nc.scalar.copy(w1b[:, :], w1p_sb[:, :])
        nc.scalar.copy(own3b[:, :], pts_own3_sb[:, :])
        nc.scalar.copy(pts1b[:, :], pts1_sb[0:3, :])

        big = st.enter_context(tc.tile_pool(name="big", bufs=1))
        hp1_hi = big.tile([128, NT1, H, CP1], BF16)
        hp1_lo = big.tile([128, NT1, H, CP1], BF16)
        es41 = big.tile([128, NT1, H], BF16)
        hp_hi = big.tile([128, NS, H, CP1], BF16)
        hp_lo = big.tile([128, NS, H, CP1], BF16)
        es4 = big.tile([128, NS, H], F32)
        ed_b = big.tile([128, H, IC], BF16)
        x_sb = big.tile([128, ICT, HC], F32)
        x_bf = big.tile([128, ICT, HC], BF16)
        xt_own = big.tile([128, 2, IC], BF16)
        edt_sb = big.tile([H, IC], BF16)
        edt_rows = big.tile([1, H, IC], BF16)
        logit_sb = big.tile([128, ICT, 2], F32)
        mn_sb = big.tile([128, NS, IC], BF16)   # layer-2 slot mask offsets

        nc.vector.memset(hp1_hi[:, :, :, C:CP1], 1.0)
        nc.vector.memset(hp1_lo[:, :, :, C:CP1], 0.0)
        nc.vector.memset(hp_hi[:, :, :, C:CP1], 1.0)
        nc.vector.memset(hp_lo[:, :, :, C:CP1], 0.0)

        # ================= layer 1 (per destination chunk) ==============
        # ---- own-column side: hT(own), edT, ED broadcasts ----
        with tc.tile_pool(name="prep1", bufs=2) as prep, \
             tc.tile_pool(name="prep_ps1", bufs=1, space="PSUM") as prep_ps:
            ht_own = prep.tile([128, 2, IC], BF16, tag="ht", bufs=1)
            for oc in range(2):
                ht_ps = prep_ps.tile([128, IC], F32, tag="ht_ps", bufs=1,
                                     name=f"ht_ps_1_{oc}")
                for lo, sz in ((0, 512), (512, 256)):
                    sl = slice(lo, lo + sz)
                    nc.tensor.matmul(
                        ht_ps[:, sl], w1b[:, oc * 128:(oc + 1) * 128],
                        own3b[:, sl], start=True, stop=True)
                nc.scalar.copy(ht_own[:, oc, :], ht_ps[:, :])

            edt_ps = prep_ps.tile([H, IC], F32, tag="edt", bufs=1)
            for s in range(2):
                for lo, sz in ((0, 512), (512, 256)):
                    sl = slice(lo, lo + sz)
                    nc.tensor.matmul(edt_ps[:, sl], adm1_sb[:, s, :],
                                     ht_own[:, s, sl],
                                     start=(s == 0), stop=(s == 1))
            nc.scalar.copy(edt_sb[:, :], edt_ps[:, :])
            for h in range(H):
                nc.sync.dma_start(out=edt_rows[0:1, h, :],
                                  in_=edt_sb[h:h + 1, :])
            # stagger: first chunks' columns land before the full rows so
            # the chunk loop can start scoring sooner
            for h in range(H):
                nc.gpsimd.partition_broadcast(ed_b[:, h, 0:256],
                                              edt_rows[0:1, h, 0:256])
            for h in range(H):
                nc.gpsimd.partition_broadcast(ed_b[:, h, 256:IC],
                                              edt_rows[0:1, h, 256:IC])

        # ---- chunk loop: h+es, per-pair masks, merged scores, agg ----
        with tc.tile_pool(name="agg_ps1", bufs=1, space="PSUM") as agg_ps:
            aggp = [agg_ps.tile([128, 2, H, C], F32, tag=f"agg{p}",
                                name=f"agg_1_{p}")
                    for p in range(ICT // 2)]
            den_ps = agg_ps.tile([128, ICT, H], F32, tag="den",
                                 name="den_1")
            with tc.tile_pool(name="jl1", bufs=3) as jl, \
                 tc.tile_pool(name="h_ps1", bufs=2, space="PSUM") as h_psp:
                # pass 1: per-tile h/es/hi-lo/masks (independent of ed_b,
                # fills the DVE queue while the prep broadcasts run)
                mnCs = []
                tbase = 0
                for ci in range(ICT):
                    Tc = t1[ci]
                    csl = slice(ci * 128, (ci + 1) * 128)
                    mnC = jl.tile([128, T1MAX, 128], BF16, tag="mn",
                                  bufs=ICT, name=f"mn1_{ci}")
                    mnCs.append(mnC)
                    for t in range(Tc):
                        s = tbase + t
                        ssl = slice(s * 128, (s + 1) * 128)
                        h_ps = h_psp.tile([128, HCE], F32, tag="h",
                                          name=f"h1_{s}")
                        nc.tensor.matmul(h_ps[:, :], pts1b[:, ssl],
                                         w1b[:, :],
                                         start=True, stop=True)
                        nc.vector.tensor_scalar_add(
                            es41[:, s, :], h_ps[:, HC:HCE], 0.0)
                        nc.scalar.copy(
                            hp1_hi[:, s, :, 0:C],
                            h_ps[:, 0:HC].rearrange("p (h c) -> p h c",
                                                    h=H))
                        nc.vector.tensor_tensor(
                            hp1_lo[:, s, :, 0:C],
                            h_ps[:, 0:HC].rearrange("p (h c) -> p h c",
                                                    h=H),
                            hp1_hi[:, s, :, 0:C], OP.subtract)
                        g_ps = h_psp.tile([128, 128], F32, tag="g",
                                          bufs=2, name=f"g1_{s}")
                        nc.tensor.matmul(g_ps[:, :], pts1_sb[:, ssl],
                                         pts_own5_sb[:, csl],
                                         start=True, stop=True)
                        nc.vector.tensor_scalar(
                            mnC[:, t, :], g_ps[:, :], R2, MNEG,
                            OP.is_ge, OP.mult)
                    tbase += Tc

                # pass 2: merged scores + aggregation per chunk
                tbase = 0
                for ci in range(ICT):
                    Tc = t1[ci]
                    csl = slice(ci * 128, (ci + 1) * 128)
                    mnC = mnCs[ci]
                    L = jl.tile([128, Tc, H, 128], BF16, tag="L4",
                                name=f"L1_{ci}")
                    e1 = ed_b[:, :, csl].rearrange(
                        "p (o h) d -> p o h d", o=1)
                    e2 = es41[:, tbase:tbase + Tc, :].rearrange(
                        "p t (h o) -> p t h o", o=1)
                    b1, b2 = bass.broadcast_tensor_aps(e1, e2)
                    nc.vector.tensor_tensor(L[:, :, :, :], b1, b2, OP.add)
                    T4 = jl.tile([128, Tc, H, 128], BF16, tag="T4",
                                 name=f"T4_1_{ci}")
                    nc.vector.tensor_scalar(T4[:, :, :, :], L[:, :, :, :],
                                            0.2, None, OP.mult)
                    nc.vector.tensor_tensor(L[:, :, :, :], L[:, :, :, :],
                                            T4[:, :, :, :], OP.max)
                    mb = mnC[:, 0:Tc, :].rearrange(
                        "p t (o d) -> p t o d", o=1)
                    bl, bm = bass.broadcast_tensor_aps(L[:, :, :, :], mb)
                    nc.vector.tensor_tensor(L[:, :, :, :], bl, bm, OP.add)
                    A = jl.tile([128, Tc, H, 128], BF16, tag="A4",
                                name=f"A1_{ci}")
                    if ci == ICT - 1:
                        nc.scalar.activation(A[:, 0:1, :, :],
                                             L[:, 0:1, :, :], AF.Exp)
                        nc.scalar.activation(A[:, 1:Tc, :, :],
                                             L[:, 1:Tc, :, :], AF.Exp)
                    else:
                        nc.scalar.activation(A[:, :, :, :], L[:, :, :, :],
                                             AF.Exp)

                    bank = aggp[ci // 2]
                    half = ci % 2
                    for t in range(Tc):
                        s = tbase + t
                        for h in range(H):
                            first = (half == 0 and t == 0 and h == 0)
                            last = (half == 1 and t == Tc - 1
                                    and h == H - 1)
                            nc.tensor.matmul(
                                bank[:, half, h, :], A[:, t, h, :],
                                hp1_hi[:, s, h, 0:C],
                                start=first, stop=False)
                            nc.tensor.matmul(
                                bank[:, half, h, :], A[:, t, h, :],
                                hp1_lo[:, s, h, 0:C],
                                start=False, stop=last)
                            nc.tensor.matmul(
                                den_ps[:, ci, h:h + 1], A[:, t, h, :],
                                hp1_hi[:, s, h, C:CP1],
                                start=(ci == 0 and t == 0 and h == 0),
                                stop=(ci == ICT - 1 and t == Tc - 1
                                      and h == H - 1))
                    tbase += Tc

            # ---- finalize x1, AllGather, transposes, layer-2 masks ----
            with tc.tile_pool(name="fin1", bufs=2) as fin, \
                 tc.tile_pool(name="fin_ps1", bufs=2,
                              space="PSUM") as fin_ps:
                qb = BP // 128
                for ic in range(ICT):
                    rec = fin.tile([128, H], F32, tag="rec",
                                   name=f"rec_1_{ic}")
                    nc.vector.reciprocal(rec[:, :], den_ps[:, ic, :])
                    for h in range(H):
                        nc.vector.scalar_tensor_tensor(
                            x_sb[:, ic, h * C:(h + 1) * C],
                            aggp[ic // 2][:, ic % 2, h, :],
                            rec[:, h:h + 1],
                            bias1_sb[:, h * C:(h + 1) * C],
                            OP.mult, OP.add)
                    if ic == qb - 1:
                        # boundary chunks done: fire the AllGather now,
                        # before the rest of the finalize
                        nc.vector.tensor_scalar(
                            x_sb[:, 0:qb, :], x_sb[:, 0:qb, :],
                            0.0, None, OP.max)
                        nc.scalar.copy(x_bf[:, 0:qb, :],
                                       x_sb[:, 0:qb, :])
                        nc.sync.dma_start(
                            out=x_bounce.rearrange("(q p) c -> p q c",
                                                   p=128),
                            in_=x_bf[:, 0:qb, :])
                        if fake_ag:
                            for r in range(n_cores):
                                nc.sync.dma_start(
                                    out=ag_out[r * BP:(r + 1) * BP, :],
                                    in_=x_bounce[:, :])
                        else:
                            nc.gpsimd.collective_compute(
                                "AllGather", OP.bypass,
                                replica_groups=[list(range(n_cores))],
                                ins=[x_bounce.opt()],
                                outs=[ag_out.opt()])
                nc.vector.tensor_scalar(x_sb[:, qb:, :], x_sb[:, qb:, :],
                                        0.0, None, OP.max)
                # transposed own x, needed for layer-2 prep
                for ic in range(ICT):
                    for oc in range(2):
                        t_ps = fin_ps.tile([128, 128], F32, tag="t_ps",
                                           name=f"t_ps_{ic}_{oc}")
                        nc.tensor.transpose(
                            t_ps[:, :],
                            x_sb[:, ic, oc * 128:(oc + 1) * 128],
                            ident_sb[:, :])
                        nc.scalar.copy(
                            xt_own[:, oc, ic * 128:(ic + 1) * 128],
                            t_ps[:, :])
                # layer-2 slot masks from static coords: fills the
                # AllGather bubble
                with tc.tile_pool(name="mn_ps2", bufs=1,
                                  space="PSUM") as mn_psp:
                    for s in range(NS):
                        for lo, sz in ((0, 512), (512, 256)):
                            sl = slice(lo, lo + sz)
                            g2 = mn_psp.tile([128, sz], F32,
                                             tag=f"g{lo}", bufs=1,
                                             name=f"g2_{s}_{lo}")
                            nc.tensor.matmul(
                                g2[:, :],
                                pts_sel5_sb[:, s * 128:(s + 1) * 128],
                                pts_own5_sb[:, sl],
                                start=True, stop=True)
                            nc.vector.tensor_scalar(
                                mn_sb[:, s, sl], g2[:, :], R2, MNEG,
                                OP.is_ge, OP.mult)

        # ================= layer 2 (per source slot) ====================
        with tc.tile_pool(name="prep2", bufs=2) as prep, \
             tc.tile_pool(name="prep_ps2", bufs=1, space="PSUM") as prep_ps:
            ht_own = prep.tile([128, 2, IC], BF16, tag="ht", bufs=1)
            for oc in range(2):
                ht_ps = prep_ps.tile([128, IC], F32, tag="ht_ps", bufs=1,
                                     name=f"ht_ps_2_{oc}")
                for s in range(2):
                    for lo, sz in ((0, 512), (512, 256)):
                        sl = slice(lo, lo + sz)
                        nc.tensor.matmul(
                            ht_ps[:, sl],
                            w2p_sb[:, s, oc * 128:(oc + 1) * 128],
                            xt_own[:, s, sl],
                            start=(s == 0), stop=(s == 1))
                nc.scalar.copy(ht_own[:, oc, :], ht_ps[:, :])

            edt_ps = prep_ps.tile([H, IC], F32, tag="edt", bufs=1)
            for s in range(2):
                for lo, sz in ((0, 512), (512, 256)):
                    sl = slice(lo, lo + sz)
                    nc.tensor.matmul(edt_ps[:, sl], adm2_sb[:, s, :],
                                     ht_own[:, s, sl],
                                     start=(s == 0), stop=(s == 1))
            nc.scalar.copy(edt_sb[:, :], edt_ps[:, :])
            for h in range(H):
                nc.sync.dma_start(out=edt_rows[0:1, h, :],
                                  in_=edt_sb[h:h + 1, :])
            for h in range(H):
                nc.gpsimd.partition_broadcast(ed_b[:, h, :],
                                              edt_rows[0:1, h, :])

        with tc.tile_pool(name="agg_ps2", bufs=1, space="PSUM") as agg_ps:
            aggp = [agg_ps.tile([128, 2, H, C], F32, tag=f"agg{p}",
                                name=f"agg_2_{p}")
                    for p in range(ICT // 2)]
            den_ps = agg_ps.tile([128, ICT, H], F32, tag="den",
                                 name="den_2")
            with tc.tile_pool(name="jl2", bufs=3) as jl, \
                 tc.tile_pool(name="h_ps2", bufs=2, space="PSUM") as h_psp:
                for s in range(NS):
                    # halo slots wait on the AllGather: tell the static
                    # scheduler to order their chain after the own slots
                    # so in-order engine queues don't stall behind it
                    if s >= OWN_SLOTS:
                        tc.tile_set_cur_wait(0.5)
                    # --- h + es for this slot's 128 sources ---
                    h_ps = h_psp.tile([128, HCE], F32, tag="h",
                                      name=f"h_ps_2_{s}")
                    if s < OWN_SLOTS:
                        for half in range(2):
                            nc.tensor.matmul(
                                h_ps[:, :],
                                xt_own[:, half, s * 128:(s + 1) * 128],
                                w2p_sb[:, half, :],
                                start=(half == 0), stop=(half == 1))
                    else:
                        xg = jl.tile([128, HC], BF16, tag="xg",
                                     name=f"xg_2_{s}")
                        nc.gpsimd.indirect_dma_start(
                            out=xg[:, :], out_offset=None,
                            in_=ag_out,
                            in_offset=bass.IndirectOffsetOnAxis(
                                ap=agidx_sb[:, s - OWN_SLOTS:
                                            s - OWN_SLOTS + 1],
                                axis=0))
                        xtg = jl.tile([128, 2, 128], BF16, tag="xtg",
                                      name=f"xtg_2_{s}")
                        for half in range(2):
                            t_ps = h_psp.tile(
                                [128, 128], BF16, tag=f"tr{half}",
                                bufs=1, name=f"tr_2_{s}_{half}")
                            nc.tensor.transpose(
                                t_ps[:, :],
                                xg[:, half * 128:(half + 1) * 128],
                                ident_bf[:, :])
                            nc.scalar.copy(xtg[:, half, :], t_ps[:, :])
                        for half in range(2):
                            nc.tensor.matmul(
                                h_ps[:, :], xtg[:, half, :],
                                w2p_sb[:, half, :],
                                start=(half == 0), stop=(half == 1))
                    nc.vector.tensor_scalar_add(es4[:, s, :],
                                                h_ps[:, HC:HCE], 0.0)
                    # h -> bf16 hi + lo with ones/zeros column
                    nc.scalar.copy(
                        hp_hi[:, s, :, 0:C],
                        h_ps[:, 0:HC].rearrange("p (h c) -> p h c", h=H))
                    nc.vector.tensor_tensor(
                        hp_lo[:, s, :, 0:C],
                        h_ps[:, 0:HC].rearrange("p (h c) -> p h c", h=H),
                        hp_hi[:, s, :, 0:C], OP.subtract)

                    # --- scores: L = leaky(ed+es) + mn ; A = exp(L) ---
                    L4 = jl.tile([128, H, IC], BF16, tag="L4",
                                 name=f"L4_2_{s}")
                    nc.scalar.activation(L4[:, 0, :], ed_b[:, 0, :],
                                         AF.Prelu,
                                         bias=es4[:, s, 0:1],
                                         scale=1.0, alpha=0.2)
                    T4 = jl.tile([128, 3, IC], BF16, tag="T4",
                                 name=f"T4_2_{s}")
                    for h in range(1, H):
                        nc.vector.tensor_scalar(
                            L4[:, h, :], ed_b[:, h, :],
                            es4[:, s, h:h + 1], None, OP.add)
                    nc.vector.tensor_scalar(
                        T4[:, :, :], L4[:, 1:4, :], 0.2, None, OP.mult)
                    nc.vector.tensor_tensor(L4[:, 1:4, :], L4[:, 1:4, :],
                                            T4[:, :, :], OP.max)
                    l4b, mnb = bass.broadcast_tensor_aps(
                        L4[:, :, :],
                        mn_sb[:, s:s + 1, :])
                    nc.vector.tensor_tensor(L4[:, :, :], l4b, mnb,
                                            OP.add)
                    A4 = jl.tile([128, H, IC], BF16, tag="A4",
                                 name=f"A4_2_{s}")
                    if s >= NS - 2:
                        nc.scalar.activation(A4[:, 0:2, :], L4[:, 0:2, :],
                                             AF.Exp)
                        nc.scalar.activation(A4[:, 2:4, :], L4[:, 2:4, :],
                                             AF.Exp)
                    else:
                        nc.scalar.activation(A4[:, :, :], L4[:, :, :],
                                             AF.Exp)

                    # --- aggregation ---
                    for h in range(H):
                        for ic in range(ICT):
                            out_ap = aggp[ic // 2][:, ic % 2, h, :]
                            first = (s == 0 and h == 0 and ic % 2 == 0)
                            last = (s == NS - 1 and h == H - 1
                                    and ic % 2 == 1)
                            nc.tensor.matmul(
                                out_ap,
                                A4[:, h, ic * 128:(ic + 1) * 128],
                                hp_hi[:, s, h, 0:C],
                                start=first, stop=False)
                            nc.tensor.matmul(
                                out_ap,
                                A4[:, h, ic * 128:(ic + 1) * 128],
                                hp_lo[:, s, h, 0:C],
                                start=False, stop=last)
                            nc.tensor.matmul(
                                den_ps[:, ic, h:h + 1],
                                A4[:, h, ic * 128:(ic + 1) * 128],
                                hp_hi[:, s, h, C:CP1],
                                start=(s == 0 and h == 0 and ic == 0),
                                stop=(s == NS - 1 and h == H - 1
                                      and ic == ICT - 1))
                tc.cur_wait_ts = None

            # ---- finalize x2 = relu(num/den + b); fc ----
            with tc.tile_pool(name="fin2", bufs=2) as fin:
                for ic in range(ICT):
                    rec = fin.tile([128, H], F32, tag="rec",
                                   name=f"rec_2_{ic}")
                    nc.vector.reciprocal(rec[:, :], den_ps[:, ic, :])
                    for h in range(H):
                        nc.vector.scalar_tensor_tensor(
                            x_sb[:, ic, h * C:(h + 1) * C],
                            aggp[ic // 2][:, ic % 2, h, :],
                            rec[:, h:h + 1],
                            bias2_sb[:, h * C:(h + 1) * C],
                            OP.mult, OP.add)
                nc.vector.tensor_scalar(x_sb[:, :, :], x_sb[:, :, :],
                                        0.0, None, OP.max)
                for ic in range(ICT):
                    for o in range(2):
                        prod = fin.tile([128, HC], F32, tag="prod",
                                        name=f"prod_{ic}_{o}")
                        nc.vector.tensor_tensor(
                            prod[:, :], x_sb[:, ic, :],
                            fcw_sb[:, o * HC:(o + 1) * HC], OP.mult)
                        red = fin.tile([128, 1], F32, tag="red",
                                       name=f"red_{ic}_{o}")
                        nc.vector.tensor_reduce(
                            red[:, :], prod[:, :], AX.X, OP.add)
                        nc.vector.tensor_scalar_add(
                            logit_sb[:, ic, o:o + 1], red[:, :],
                            fcb_sb[:, o:o + 1])
                nc.sync.dma_start(
                    out=out_d.rearrange("(q p) o -> p q o", p=128),
                    in_=logit_sb[:, :, :])

    nc.compile()
    return nc


_BUILD_CACHE = {}


def _get_nc(nslot, bp, t1):
    key = (nslot, bp, t1)
    if key not in _BUILD_CACHE:
        _BUILD_CACHE[key] = build(nslot, bp, t1)
    return _BUILD_CACHE[key]


def _morton(p, bits=10):
    q = np.clip((p * (1 << bits)).astype(np.int64), 0, (1 << bits) - 1)
    code = np.zeros(len(p), np.int64)
    for b in range(bits):
        for dim in range(3):
            code |= ((q[:, dim] >> b) & 1) << (3 * b + dim)
    return code


def _plan(pts):
    """Sort nodes spatially, build compacted source lists for both layers."""
    order = np.argsort(_morton(pts), kind="stable")
    p_sorted = np.full((KP, 3), PAD_COORD, np.float32)
    p_sorted[:K] = pts[order]

    sq = (p_sorted ** 2).sum(-1, dtype=np.float32)
    G = p_sorted @ p_sorted.T
    d2 = sq[None, :] + sq[:, None] - 2.0 * G
    near = d2 < (R2 + MASK_EPS)          # [j, i], conservative superset

    halos = []
    for c in range(N_CORES):
        act = np.flatnonzero(near[:, c * IC:(c + 1) * IC].any(axis=1))
        halos.append(act[(act < c * IC) | (act >= (c + 1) * IC)])
    nslot = max(OWN_SLOTS + 1,
                max(OWN_SLOTS + (len(h) + 127) // 128 for h in halos))
    # permute each core's own nodes boundary-first so the AllGather only
    # ships the rows other cores actually fetch
    boundary = np.zeros(KP, bool)
    for h in halos:
        boundary[h] = True
    lists, agpos = [], np.zeros(KP, np.int64)
    bnd_counts = []
    for c in range(N_CORES):
        own = np.arange(c * IC, (c + 1) * IC)
        isb = boundary[own]
        perm = np.concatenate([own[isb], own[~isb]])
        bnd_counts.append(int(isb.sum()))
        lists.append(perm)
    bp = max(128, 128 * ((max(bnd_counts) + 127) // 128))
    for c in range(N_CORES):
        agpos[lists[c][:bnd_counts[c]]] = c * bp + np.arange(bnd_counts[c])
    # layer-1 per-chunk compacted source tiles (chunks in permuted order)
    raw1 = []
    for c in range(N_CORES):
        per_chunk = []
        for ci in range(ICT):
            dst = lists[c][ci * 128:(ci + 1) * 128]
            per_chunk.append(np.flatnonzero(near[:, dst].any(axis=1)))
        raw1.append(per_chunk)
    t1 = tuple(max(128, (max(len(raw1[c][ci]) for c in range(N_CORES))
                         + 127) // 128 * 128) // 128
               for ci in range(ICT))
    l1sels = []
    for c in range(N_CORES):
        parts = []
        for ci in range(ICT):
            src = raw1[c][ci]
            parts.append(np.concatenate(
                [src, np.full(t1[ci] * 128 - len(src), PAD_NODE,
                              src.dtype)]))
        l1sels.append(np.concatenate(parts))
    for c in range(N_CORES):
        l = np.concatenate([lists[c], halos[c]])
        lists[c] = np.concatenate(
            [l, np.full(nslot * 128 - len(l), PAD_NODE, l.dtype)])
    return order, p_sorted, lists, nslot, agpos, bp, l1sels, t1


def _prep_inputs(pos, pos_non_manifold, W1, a_src1, a_dst1, b1,
                 W2, a_src2, a_dst2, b2, fc_w, fc_b):
    bf16 = ml_dtypes.bfloat16
    pts = np.concatenate([np.asarray(pos, np.float32),
                          np.asarray(pos_non_manifold, np.float32)],
                         axis=2)[0].T  # [K, 3]
    order, p_sorted, lists, nslot, agpos, bp, l1sels, t1 = _plan(pts)
    sq_sorted = (p_sorted ** 2).sum(-1, dtype=np.float32).astype(np.float32)

    def bcast128(v):
        v = np.asarray(v, np.float32).reshape(-1)
        return np.ascontiguousarray(
            np.broadcast_to(v[None, :], (128, v.size)))

    def blockdiag(a):  # [H, C] -> [HC, H] fp32
        m = np.zeros((HC, H), dtype=np.float32)
        for h in range(H):
            m[h * C:(h + 1) * C, h] = np.asarray(a, np.float32)[h]
        return m

    def sel5_of(sel):
        psel = p_sorted[sel]
        return np.ascontiguousarray(np.concatenate(
            [psel.T, sq_sorted[sel][None, :],
             np.ones((1, len(sel)), np.float32)], axis=0)
            .astype(np.float32))

    W1f = np.asarray(W1, np.float32)
    W2f = np.asarray(W2, np.float32)
    w1p = np.concatenate([W1f, W1f @ blockdiag(a_src1)], axis=1)
    w2p = np.concatenate([W2f, W2f @ blockdiag(a_src2)], axis=1)

    shared = {
        "w1p": np.ascontiguousarray(w1p.astype(np.float32)),
        "w2p": np.ascontiguousarray(w2p.astype(bf16)),
        "adm1": blockdiag(a_dst1).astype(bf16),
        "adm2": blockdiag(a_dst2).astype(bf16),
        "bias1": bcast128(b1),
        "bias2": bcast128(b2),
        "fcw": bcast128(np.asarray(fc_w, np.float32).T),
        "fcb": bcast128(fc_b),
        "ident": np.eye(128, dtype=np.float32),
    }
    in_maps = []
    for c in range(N_CORES):
        sel = lists[c]
        pown = p_sorted[sel[:IC]]                 # own nodes, boundary-first
        own5 = np.concatenate(
            [-2.0 * pown.T, np.ones((1, IC), np.float32),
             (pown ** 2).sum(-1, dtype=np.float32)[None, :]], axis=0)
        # halo slots index boundary-layout ag rows via the host map
        agidx = np.ascontiguousarray(
            agpos[sel[OWN_SLOTS * 128:]].reshape(-1, 128).T
            .astype(np.int32))
        m = dict(shared)
        m["pts_sel5"] = sel5_of(sel)
        m["pts1"] = sel5_of(l1sels[c])
        m["pts_own5"] = np.ascontiguousarray(own5.astype(np.float32))
        m["pts_own3"] = np.ascontiguousarray(pown.T)
        m["agidx"] = agidx
        in_maps.append(m)
    return in_maps, order, nslot, lists, bp, t1


def kernel(pos, pos_non_manifold, W1, a_src1, a_dst1, b1,
           W2, a_src2, a_dst2, b2, fc_w, fc_b, _trace=False):
    in_maps, order, nslot, lists, bp, t1 = _prep_inputs(
        pos, pos_non_manifold, W1, a_src1, a_dst1, b1,
        W2, a_src2, a_dst2, b2, fc_w, fc_b)
    nc = _get_nc(nslot, bp, t1)
    res = run_bass_kernel_spmd(nc, in_maps, core_ids=list(range(N_CORES)),
                               trace=_trace)
    kernel.last_results = res
    x2s = np.concatenate([res.results[c]["out"] for c in range(N_CORES)],
                         axis=0)  # [KP, 2], rows in per-core list order
    perm = np.concatenate([lists[c][:IC] for c in range(N_CORES)])
    x2p = np.empty((KP, 2), np.float32)
    x2p[perm] = x2s
    x2 = np.empty((K, 2), np.float32)
    x2[order] = x2p[:K]
    logits = np.ascontiguousarray(x2[M:K]).reshape(1, 2, 3000)
    return logits.astype(np.float32)
